# revision 1
# baseline (speedup 1.0000x reference)
"""Trainium2 Bass kernel for nn_CNN_Mem (CNN text encoder + cosine memory lookup).

Strategy (8 NeuronCores, SPMD):
  - Memory bank sharded along mem_size: host label-sorts mem_keys so every
    16-column block holds a single label (groups padded by duplicating a real
    key of the same label -> maxes are exact), casts to fp16, transposes to
    [300, M/8] slabs per core.
  - Each core: CNN for its 16 batch rows (embedding rows gathered host-side,
    convs as PSUM-accumulated matmuls over shifted APs, relu+bias on ACT,
    maxpool on DVE) -> feature chunks [100, 16] per kernel size. These are
    exactly the d-chunks of q^T. AllGather across the 8 cores -> lhsT
    [100, 128] per d-chunk.
  - Stream the keysT slab through the PE in [128, 512] PSUM chunks
    (3 accumulated fp16 matmuls each), segmented reduce_max (blocks of 16)
    -> per-block maxes; then one small masked-max pass over block labels
    gives per-core sim_pos/sim_neg partials (shifted by +SHIFT so empty
    positives read as 0).
  - Host combines: max over cores, divide by feature norms (maxes commute
    with the positive per-row normalization, so the kernel works on
    unnormalized features; norms come back via a sumsq output), then
    loss = mean(relu(sim_neg - sim_pos + margin)),
    accuracy = mean(sim_pos > sim_neg)  (equivalent to the argmax form for
    distinct maxima: the nearest neighbour's label matches y iff the best
    positive beats the best negative).

Performance notes:
  - fp16 keys halve the streamed bytes (dominant cost: 262144x300 bank ->
    ~20 MB/core) and run the PE at 1 cycle/row; fp32 margin analysis shows
    |sim_pos - sim_neg| >= 0.034 per row vs ~1e-4 fp16-induced error.
  - SBUF DMA ports are bound to partition octets (even SDMA engines serve
    partitions 0-63, odd 64-127), so a [100, N] transfer runs at ~78% of
    peak. Each d-chunk's 100 rows are therefore scattered over all 128
    partitions (3 stride-4 stripes + 4 extras on distinct ports ->
    ~98.7% port balance); the matching lhsT is built on-device with a
    permutation matmul whose zero rows also nullify the never-written
    partitions of the key tiles.
  - PSUM chunk maxes are copied to SBUF as f16 by the otherwise-idle ACT
    engine so the DVE segmented reduce runs in 2-4x perf mode; label
    masks (+-16 sentinels, exact in f16) are precomputed while the CNN
    runs, leaving a ~3 us masked-max tail.
  - Conv weights are padded to 128 output channels so Fast-Weight-Load
    engages on the CNN matmuls; the feature AllGather is issued before the
    sumsq block so collective latency overlaps remaining CNN work; the
    permutation matmuls run in f32 directly on the gathered features
    (no separate cast pass).
  - Correctness vs the f32 jax reference: rel err ~6e-5 on loss, accuracy
    exact. DMA-busy floor for the fp16 stream is ~59 us/core at balanced
    ports; cost-model (port/FWL-blind) says ~0.1-0.15 ms.
"""
import numpy as np
from contextlib import ExitStack

import concourse.bass as bass
import concourse.tile as tile
from concourse import bacc, mybir
from concourse.bass_utils import run_bass_kernel_spmd

# ---- problem dims (hardcoded; harness passes matching inputs) ----
B, L = 128, 64
V, D = 25000, 300
C = 1000
KN = 100
KSIZES = (3, 4, 5)
M, KEY = 262144, 300
MARGIN = 0.1

N_CORES = 8
BPC = B // N_CORES          # batch rows per core
TOK = BPC * L               # tokens per core
DCN = 3                     # d-chunks of 100
DCW = 100                   # d-chunk width
KNP = 128                   # conv output channels padded for FWL
CHUNK = 512                 # sim columns per PSUM chunk
BLK = 16                    # label-pure block width
NCH = 66                    # chunks per core
G = 6                       # chunks per DMA group
W = NCH * CHUNK             # slab columns per core (33792)
CAP = N_CORES * W           # padded memory size (270336)
NBLK = W // BLK             # blocks per core (2112)
BIG = 16.0                  # mask sentinel; |sims_u| <= ~8 so +-16 acts as +-inf

f32 = mybir.dt.float32
f16 = mybir.dt.float16

# port-balanced partition scatter: each d-chunk's 100 rows live on
# partitions {p : p%4 < 3} (96 stripe rows) plus 4 extras on distinct
# DMA ports per chunk; remaining rows are zeros in the stationary operand.
XTRA = (3, 7, 67)           # extras offset per d-chunk (step 8, count 4)


def _scatter_partitions(dc):
    ps = [4 * a + i for a in range(32) for i in range(3)]
    ps += [XTRA[dc] + 8 * j for j in range(4)]
    return ps

_CACHED_NC = None


def build(collective=True, g=11, ktbufs=4, skip_cnn=False, balanced=True):
    nc = bacc.Bacc("TRN2", target_bir_lowering=False, debug=False,
                   num_devices=N_CORES if collective else 1)
    qt_in = None
    if not collective:
        qt_in = nc.declare_dram_parameter("qtin", [DCN, DCW, B], f16,
                                          isOutput=False)

    kt_in = [nc.declare_dram_parameter(f"kt{c}", [DCW, W], f16, isOutput=False)
             for c in range(DCN)]
    et_in = nc.declare_dram_parameter("et", [DCN, DCW, TOK], f16,
                                      isOutput=False)
    wt_in = {k: nc.declare_dram_parameter(f"wt{k}", [DCW, k * DCN * KNP], f16,
                                          isOutput=False) for k in KSIZES}
    bias_in = {k: nc.declare_dram_parameter(f"bias{k}", [KNP, 1], f32,
                                            isOutput=False) for k in KSIZES}
    pm_in = [nc.declare_dram_parameter(f"pm{c}", [DCW, B], f32, isOutput=False)
             for c in range(DCN)] if balanced else None
    y_in = nc.declare_dram_parameter("yv", [B, 1], f32, isOutput=False)
    blab_in = nc.declare_dram_parameter("blab", [1, NBLK], f16, isOutput=False)

    pos_out = nc.declare_dram_parameter("pos", [B, 1], f32, isOutput=True)
    neg_out = nc.declare_dram_parameter("neg", [B, 1], f32, isOutput=True)
    ss_out = nc.declare_dram_parameter("ss", [1, BPC], f32, isOutput=True)

    cc_in = nc.dram_tensor("cc_in", [B, DCN * BPC], f16)
    cc_out = nc.dram_tensor("cc_out", [N_CORES, B, DCN * BPC], f16,
                            addr_space="Shared")

    with tile.TileContext(nc) as tc, ExitStack() as ctx:
        singles = ctx.enter_context(tc.tile_pool(name="singles", bufs=1))
        ktp = ctx.enter_context(tc.tile_pool(name="ktp", bufs=ktbufs))
        work = ctx.enter_context(tc.tile_pool(name="work", bufs=1))

        # ---------------- CNN phase ----------------
        et = []
        for dc in range(DCN):
            t = singles.tile([DCW, TOK], f16, name=f"et{dc}", tag=f"et{dc}")
            nc.sync.dma_start(out=t, in_=et_in[dc, :, :])
            et.append(t)
        wt = {}
        bia = {}
        for k in KSIZES:
            wt[k] = singles.tile([DCW, k * DCN * KNP], f16, name=f"wt{k}", tag=f"wt{k}")
            nc.sync.dma_start(out=wt[k], in_=wt_in[k][:, :])
            bia[k] = singles.tile([KNP, 1], f32, name=f"bias{k}", tag=f"bias{k}")
            nc.sync.dma_start(out=bia[k], in_=bias_in[k][:, :])

        feats = {}  # per kernel size: [100, BPC] f32 (this IS qT d-chunk)
        with tc.tile_pool(name="cnnps", bufs=2, space="PSUM") as cnnps, \
             tc.tile_pool(name="cnnsb", bufs=2) as cnnsb:
          if skip_cnn:
            ss_sb = singles.tile([1, BPC], f32, tag="ss_sb")
            nc.vector.memset(ss_sb, 0.0)
            nc.sync.dma_start(out=ss_out[:, :], in_=ss_sb[:])
          else:
              for k in KSIZES:
                  lout = L - k + 1
                  half = BPC // 2
                  fk = singles.tile([KNP, BPC], f32, name=f"feats{k}", tag=f"feats{k}")
                  feats[k] = fk
                  for h in range(2):
                      ps = cnnps.tile([KNP, half * lout], f32, tag="cnnpsum")
                      first = True
                      for t in range(k):
                          for dc in range(DCN):
                              rhs = et[dc].rearrange(
                                  "p (b l) -> p b l", l=L)[:, h * half:(h + 1) * half,
                                                           t:t + lout]
                              nc.tensor.matmul(
                                  ps[:],
                                  wt[k][:, (t * DCN + dc) * KNP:(t * DCN + dc + 1) * KNP],
                                  rhs,
                                  start=first, stop=(t == k - 1 and dc == DCN - 1))
                              first = False
                      # bias + relu (ACT), then maxpool over positions (DVE)
                      rk = cnnsb.tile([KNP, half * lout], f32, tag="relu")
                      nc.scalar.activation(rk[:], ps[:],
                                           mybir.ActivationFunctionType.Relu,
                                           bias=bia[k][:], scale=1.0)
                      nc.vector.tensor_reduce(
                          out=fk[:, h * half:(h + 1) * half],
                          in_=rk.rearrange("p (b l) -> p b l", l=lout),
                          axis=mybir.AxisListType.X, op=mybir.AluOpType.max)

              # perm-scatter local features across all 128 partitions,
              # then AllGather f16 already in the stream-lhsT layout: the
              # post-collective chain is just one readback DMA per d-chunk
              if collective:
                  fall = singles.tile([B, DCN * BPC], f16, tag="fall")
                  if balanced:
                      for i, k in enumerate(KSIZES):
                          pm = singles.tile([DCW, B], f32, name=f"pm{i}",
                                            tag=f"pm{i}")
                          nc.sync.dma_start(out=pm, in_=pm_in[i][:, :])
                          pq = cnnps.tile([B, BPC], f32, tag="pq")
                          nc.tensor.matmul(pq[:], pm[:], feats[k][:DCW, :],
                                           start=True, stop=True)
                          nc.vector.tensor_copy(
                              fall[:, i * BPC:(i + 1) * BPC], pq[:])
                  else:
                      nc.vector.memset(fall, 0.0)
                      for i, k in enumerate(KSIZES):
                          nc.vector.tensor_copy(
                              fall[:DCW, i * BPC:(i + 1) * BPC],
                              feats[k][:DCW, :])
                  nc.sync.dma_start(out=cc_in[:, :], in_=fall[:])
                  nc.gpsimd.collective_compute(
                      "AllGather", mybir.AluOpType.bypass,
                      replica_groups=[list(range(N_CORES))],
                      ins=[cc_in[:, :]], outs=[cc_out[:, :, :]])

              # sumsq of features per local batch row: ss[1, BPC]
              ones = singles.tile([DCW, 1], f32, tag="ones")
              nc.vector.memset(ones, 1.0)
              ssps = cnnps.tile([1, BPC], f32, tag="ssps")
              for i, k in enumerate(KSIZES):
                  sq = cnnsb.tile([DCW, BPC], f32, tag="sq")
                  nc.vector.tensor_mul(sq[:], feats[k][:DCW, :], feats[k][:DCW, :])
                  nc.tensor.matmul(ssps[:], ones[:], sq[:],
                                   start=(i == 0), stop=(i == len(KSIZES) - 1))
              ss_sb = singles.tile([1, BPC], f32, tag="ss_sb")
              nc.vector.tensor_copy(ss_sb[:], ssps[:])
              nc.sync.dma_start(out=ss_out[:, :], in_=ss_sb[:])

        # ---------------- AllGather readback ----------------
        qt = []
        if collective:
            for dc in range(DCN):
                qs = singles.tile([B, N_CORES, BPC], f16,
                                  name=f"qts{dc}", tag=f"qts{dc}")
                src = bass.AP(tensor=cc_out.ap().tensor,
                              offset=dc * BPC,
                              ap=[[DCN * BPC, B], [B * DCN * BPC, N_CORES],
                                  [1, BPC]])
                nc.sync.dma_start(out=qs, in_=src)
                full = qs.rearrange("p a b -> p (a b)")
                qt.append(full if balanced else full[:DCW, :])
        else:
            with tc.tile_pool(name="qperm", bufs=2, space="PSUM") as qpp:
                for dc in range(DCN):
                    q16 = singles.tile([DCW, B], f16, name=f"q16_{dc}",
                                       tag=f"q16_{dc}")
                    nc.sync.dma_start(out=q16, in_=qt_in[dc, :, :])
                    if balanced:
                        qf = singles.tile([DCW, B], f32, name=f"qf{dc}",
                                          tag=f"qf{dc}")
                        nc.vector.tensor_copy(qf[:], q16[:])
                        pm = singles.tile([DCW, B], f32, name=f"pmq{dc}",
                                          tag=f"pmq{dc}")
                        nc.sync.dma_start(out=pm, in_=pm_in[dc][:, :])
                        pq = qpp.tile([B, B], f32, tag="pq2")
                        nc.tensor.matmul(pq[:], pm[:], qf[:],
                                         start=True, stop=True)
                        qs = singles.tile([B, B], f16, name=f"qts{dc}",
                                          tag=f"qts{dc}")
                        nc.vector.tensor_copy(qs[:], pq[:])
                        qt.append(qs)
                    else:
                        qt.append(q16)

        # ---------------- mask prep (early; independent of stream) -------
        blab_b = work.tile([B, NBLK], f16, tag="blab_b")
        nc.sync.dma_start(out=blab_b, in_=bass.AP(
            tensor=blab_in.ap().tensor, offset=0, ap=[[0, B], [1, NBLK]]))
        y0 = singles.tile([B, 1], f32, tag="y0")
        nc.sync.dma_start(out=y0, in_=y_in[:, :])
        yv = singles.tile([B, 1], f32, tag="yv")
        nc.vector.tensor_copy(yv[:], y0[:])
        eq01 = work.tile([B, NBLK], f16, tag="eq01")
        nc.vector.tensor_scalar(out=eq01[:], in0=blab_b[:], scalar1=yv[:],
                                scalar2=None, op0=mybir.AluOpType.is_equal)
        # eqp = +BIG where label==y else -BIG ; eqn = the opposite
        eqp = work.tile([B, NBLK], f16, tag="eqp")
        nc.vector.tensor_scalar(out=eqp[:], in0=eq01[:], scalar1=2.0 * BIG,
                                scalar2=-BIG, op0=mybir.AluOpType.mult,
                                op1=mybir.AluOpType.add)
        eqn = work.tile([B, NBLK], f16, tag="eqn")
        nc.vector.tensor_scalar(out=eqn[:], in0=eq01[:], scalar1=-2.0 * BIG,
                                scalar2=BIG, op0=mybir.AluOpType.mult,
                                op1=mybir.AluOpType.add)

        # ---------------- memory stream ----------------
        bmall = work.tile([B, NBLK], f16, tag="bmall")
        with tc.tile_pool(name="simps", bufs=8, space="PSUM") as simps:
            # tapered tail: finer last groups so the final chunks' data
            # lands progressively earlier, shrinking the post-DMA PE tail
            sizes = []
            left = NCH
            while left > g:
                sizes.append(g)
                left -= g
            while left > 2:
                h2 = max(2, left - (left + 1) // 2)
                sizes.append((left + 1) // 2)
                left -= sizes[-1]
            if left:
                sizes.append(left)
            starts = [sum(sizes[:i]) for i in range(len(sizes))]
            for gi, (j0, gsz) in enumerate(zip(starts, sizes)):
                gw = gsz * CHUNK
                kt = []
                for dc in range(DCN):
                    if balanced:
                        t = ktp.tile([B, g * CHUNK], f16, name=f"ktt{dc}",
                                     tag=f"kt{dc}")
                        if gi < ktbufs:
                            nc.vector.memset(t, 0.0)
                        src = kt_in[dc][:, j0 * CHUNK:j0 * CHUNK + gw]
                        for i in range(3):  # stripe rows r%3==i -> p=4a+i
                            nc.sync.dma_start(out=t[i:i + 125:4, :gw],
                                              in_=src[i:96:3, :])
                        x = XTRA[dc]
                        nc.sync.dma_start(out=t[x:x + 30:8, :gw],
                                          in_=src[96:100, :])
                    else:
                        t = ktp.tile([DCW, g * CHUNK], f16, name=f"ktt{dc}",
                                     tag=f"kt{dc}")
                        nc.sync.dma_start(
                            out=t[:, :gw],
                            in_=kt_in[dc][:, j0 * CHUNK:j0 * CHUNK + gw])
                    kt.append(t)
                pss = []
                for j in range(gw // CHUNK):
                    pss.append(simps.tile([B, CHUNK], f32, name="simpsum", tag="simpsum"))
                for dc in range(DCN):
                    for j in range(gw // CHUNK):
                        nc.tensor.matmul(
                            pss[j][:], qt[dc][:],
                            kt[dc][:, j * CHUNK:(j + 1) * CHUNK],
                            start=(dc == 0), stop=(dc == DCN - 1))
                for j in range(gw // CHUNK):
                    sc = ktp.tile([B, CHUNK], f16, name="simf16", tag="simf16")
                    nc.scalar.copy(sc[:], pss[j][:])
                    nc.vector.tensor_reduce(
                        out=bmall[:, (j0 + j) * (CHUNK // BLK):
                                  (j0 + j + 1) * (CHUNK // BLK)],
                        in_=sc.rearrange("p (nb blk) -> p nb blk", blk=BLK),
                        axis=mybir.AxisListType.X, op=mybir.AluOpType.max)

        # ---------------- masked maxes over block labels ----------------
        # quarter-sliced so the early quarters overlap the tail of the
        # stream (their bmall ranges are complete mid-stream)
        NQ = 4
        QW = NBLK // NQ
        posm = work.tile([B, NBLK], f16, tag="posm")
        negm = work.tile([B, NBLK], f16, tag="negm")
        pos4 = singles.tile([B, NQ], f32, tag="pos4")
        neg4 = singles.tile([B, NQ], f32, tag="neg4")
        for q in range(NQ):
            sl = slice(q * QW, (q + 1) * QW)
            nc.vector.tensor_tensor(out=posm[:, sl], in0=bmall[:, sl],
                                    in1=eqp[:, sl], op=mybir.AluOpType.min)
            nc.vector.tensor_reduce(out=pos4[:, q:q + 1], in_=posm[:, sl],
                                    axis=mybir.AxisListType.X,
                                    op=mybir.AluOpType.max)
            nc.vector.tensor_tensor(out=negm[:, sl], in0=bmall[:, sl],
                                    in1=eqn[:, sl], op=mybir.AluOpType.min)
            nc.vector.tensor_reduce(out=neg4[:, q:q + 1], in_=negm[:, sl],
                                    axis=mybir.AxisListType.X,
                                    op=mybir.AluOpType.max)

        pos = singles.tile([B, 1], f32, tag="pos")
        nc.vector.tensor_reduce(out=pos[:], in_=pos4[:],
                                axis=mybir.AxisListType.X,
                                op=mybir.AluOpType.max)
        nc.sync.dma_start(out=pos_out[:, :], in_=pos[:])
        neg = singles.tile([B, 1], f32, tag="neg")
        nc.vector.tensor_reduce(out=neg[:], in_=neg4[:],
                                axis=mybir.AxisListType.X,
                                op=mybir.AluOpType.max)
        nc.sync.dma_start(out=neg_out[:, :], in_=neg[:])

    nc.compile()
    return nc


def _prep(x, y, embed, conv_w3, conv_b3, conv_w4, conv_b4, conv_w5, conv_b5,
          mem_keys, mem_values):
    """Host-side sharding/packing. Returns per-core input maps + combine data."""
    x = np.asarray(x)
    y64 = np.asarray(y).astype(np.int64)
    mv = np.asarray(mem_values).astype(np.int64)
    mk = np.asarray(mem_keys, dtype=np.float32)

    # --- label-sorted, block-pure padded permutation of the memory bank ---
    order = np.argsort(mv, kind="stable")
    cnt = np.bincount(mv, minlength=C)
    assert cnt.min() > 0, "kernel assumes every class present in memory"
    starts = np.zeros(C + 1, np.int64)
    starts[1:] = np.cumsum(cnt)
    parts = []
    for c in range(C):
        g = order[starts[c]:starts[c + 1]]
        padn = (-len(g)) % BLK
        if padn:
            g = np.concatenate([g, np.repeat(g[0], padn)])
        parts.append(g)
    perm = np.concatenate(parts)
    assert len(perm) <= CAP, f"padded size {len(perm)} exceeds CAP {CAP}"
    perm = np.concatenate([perm, np.repeat(perm[0], CAP - len(perm))])
    labP = mv[perm]
    blab = labP[::BLK].astype(np.float16)          # [CAP // BLK]
    keysP = mk.astype(np.float16)[perm]            # cast before gather: half the traffic

    # --- embedding lookup (host gather; device gets ready eT slabs) ---
    emb16 = np.asarray(embed, dtype=np.float32).astype(np.float16)
    e = emb16[x]                                    # [B, L, 300]
    # eT[dc, p, b*L + l] = e[b, l, dc*100 + p]
    eT = np.ascontiguousarray(
        e.reshape(B, L, DCN, DCW).transpose(2, 3, 0, 1).reshape(DCN, DCW, B * L))

    # --- conv weights: wt[k][p, (t*3+dc)*KN + kn] = w_k[kn, dc*100+p, t] ---
    wts, biases = {}, {}
    for k, w_, b_ in ((3, conv_w3, conv_b3), (4, conv_w4, conv_b4),
                      (5, conv_w5, conv_b5)):
        w_ = np.asarray(w_, dtype=np.float32)       # [KN, D, k]
        a = w_.reshape(KN, DCN, DCW, k).transpose(3, 1, 2, 0)  # [t, dc, p, kn]
        a = a.transpose(2, 0, 1, 3)                 # [p, t, dc, kn]
        ap = np.zeros((DCW, k, DCN, KNP), np.float32)
        ap[:, :, :, :KN] = a
        wts[k] = np.ascontiguousarray(
            ap.reshape(DCW, k * DCN * KNP)).astype(np.float16)
        bp = np.zeros((KNP, 1), np.float32)
        bp[:KN, 0] = np.asarray(b_, dtype=np.float32)
        biases[k] = bp

    yv = y64.astype(np.float32).reshape(B, 1)

    # permutation matrices for the port-balanced partition scatter
    pms = []
    for dc in range(DCN):
        pm = np.zeros((DCW, B), np.float32)
        for r, p in enumerate(_scatter_partitions(dc)):
            pm[r, p] = 1.0
        pms.append(pm)

    in_maps = []
    for c in range(N_CORES):
        m = {
            "et": np.ascontiguousarray(
                eT.reshape(DCN, DCW, B, L)[:, :, c * BPC:(c + 1) * BPC, :]
                .reshape(DCN, DCW, TOK)),
            "yv": yv,
            "blab": np.ascontiguousarray(
                blab[c * NBLK:(c + 1) * NBLK]).reshape(1, NBLK),
        }
        for dc in range(DCN):
            m[f"kt{dc}"] = np.ascontiguousarray(
                keysP[c * W:(c + 1) * W, dc * DCW:(dc + 1) * DCW].T)
            m[f"pm{dc}"] = pms[dc]
        for k in KSIZES:
            m[f"wt{k}"] = wts[k]
            m[f"bias{k}"] = biases[k]
        in_maps.append(m)
    return in_maps, y64


def _combine(results, y64):
    pos = np.max([r["pos"].reshape(B) for r in results], axis=0)
    neg = np.max([r["neg"].reshape(B) for r in results], axis=0)
    ss = np.concatenate([r["ss"].reshape(BPC) for r in results])  # [B]
    rn = 1.0 / np.maximum(np.sqrt(ss), 1e-12)
    sp = pos * rn
    sn = neg * rn
    loss = np.float32(np.mean(np.maximum(sn - sp + MARGIN, 0.0)))
    acc = np.float32(np.mean((sp > sn).astype(np.float32)))
    return loss, acc


def kernel(**inputs):
    global _CACHED_NC
    in_maps, y64 = _prep(**inputs)
    if _CACHED_NC is None:
        _CACHED_NC = build()
    res = run_bass_kernel_spmd(_CACHED_NC, in_maps,
                               core_ids=list(range(N_CORES)))
    return _combine(res.results, y64)



# revision 45
# speedup vs baseline: 1.8911x; 1.8911x over previous
"""Trainium2 Bass kernel for nn_CNN_Mem (CNN text encoder + cosine memory lookup).

Strategy (8 NeuronCores, SPMD):
  - Memory bank sharded along mem_size: host label-sorts mem_keys so every
    16-column block holds a single label (per-class groups padded by
    duplicating a real key of the same class; tail padded with the last
    class), casts to fp16, transposes to [300, M/8] slabs per core, and
    packs the three 100-row d-chunks chunk-major so each stream group is
    ONE contiguous DMA.
  - Each core: CNN for its 16 batch rows (embedding rows gathered host-side,
    convs as PSUM-accumulated matmuls over shifted APs, relu+bias on ACT,
    maxpool on DVE) -> feature chunks [100, 16] per kernel size = the
    d-chunks of q^T. AllGather across the 8 cores -> qT [100, 128] per
    d-chunk (single strided readback DMA).
  - Stream the packed key slab through the PE in [128, 1024] two-bank PSUM
    super-chunks (6 accumulated fp16 matmuls); ACT copies PSUM->SBUF f16;
    DVE computes per-block (16-wide) maxes with a 4-level pairwise-max
    tree (tensor_tensor runs in 2x mode, unlike tensor_reduce) -> bmall.
  - Masked maxes at block level: block labels are DMA-broadcast across
    partitions, compared against y (is_equal) and turned into +-BIG
    sentinels; per window, min(bmall, mask) + max-reduce give per-window
    sim_pos / sim_neg partials; a final max over windows produces pos/neg.
    Windows narrow toward the end of the stream (the last two chunks run
    chunk-granular) so the closing chain after the final matmul is short.
  - Feature norms come from the gathered qT itself (ones-matmul of its
    square), so the PE queue never waits on the CNN maxpool chain.
  - Host combines: max over cores, divide by feature norms (maxes commute
    with the positive per-row normalization), then margin loss + accuracy
    (sp > sn is equivalent to the argmax form for distinct maxima).

Performance notes (TimelineSim + HW-validated instruction set):
  - fp16 keys halve streamed bytes; DMA floor ~57us/core at the modeled
    360 B/ns aggregate DMA bandwidth. fp32 margin analysis: per-row
    |sim_pos - sim_neg| >= 0.034 vs ~1e-4 fp16-induced error.
  - One DMA per stream group (packed layout) keeps the shared descriptor
    generator (HWDGE, ~0.6us per DMA) off the critical path.
  - PE p-state warmup (one long zero accumulation group) covers the input
    DMA window so CNN and stream matmuls all dispatch at full clock; its
    PSUM tile doubles as stream super 0's, dodging the CNN-pool bank WAR.
  - Queue assignment: SP carries the input stream, ACT the collective
    chain (harmless: everything behind it needs the AllGather anyway) and
    outputs. Only HW-proven instructions are used (tensor_mask_reduce /
    tensor_tensor_reduce / gpsimd ALU ops all fail neuronxcc on TRN2).
"""
import numpy as np
from contextlib import ExitStack

import concourse.bass as bass
import concourse.tile as tile
from concourse import bacc, mybir
from concourse.bass_utils import run_bass_kernel_spmd

# ---- problem dims (hardcoded; harness passes matching inputs) ----
B, L = 128, 64
V, D = 25000, 300
C = 1000
KN = 100
KSIZES = (3, 4, 5)
M, KEY = 262144, 300
MARGIN = 0.1

N_CORES = 8
BPC = B // N_CORES          # batch rows per core
TOK = BPC * L               # tokens per core
DCN = 3                     # d-chunks of 100
DCW = 100                   # d-chunk width
CHUNK = 512                 # sim columns per PSUM bank
BLK = 16                    # label-pure block width
NCH = 66                    # chunks per core
W = NCH * CHUNK             # slab columns per core (33792)
CAP = N_CORES * W           # padded memory size (270336)
NBLK = W // BLK             # blocks per core (2112)
CPB = CHUNK // BLK          # blocks per chunk (32)
BIG = 16.0                  # mask sentinel; |sims_u| <= ~8 so +-16 acts as +-inf

SW = 2                      # chunks per super-chunk (PSUM/ACT granularity)
FINE = 2                    # trailing chunks run chunk-granular (short tail)
NSUP = (NCH - FINE) // SW   # wide supers per core (32)
SUPB = SW * CPB             # blocks per super (64)
WIN_SUP = (8, 8, 8, 5, 1, 1, 1)   # mask windows, in supers (sum = NSUP)

f32 = mybir.dt.float32
f16 = mybir.dt.float16

_CACHED_NC = None


def _group_sizes(g, taper=(1, 1, 1)):
    left = NSUP - sum(taper)
    sizes = []
    while left > 0:
        s = min(g, left)
        sizes.append(s)
        left -= s
    return sizes + list(taper)


def build(collective=True, g=2, ktbufs=6, scbufs=4, warmup=15, warmw=256):
    nc = bacc.Bacc("TRN2", target_bir_lowering=False, debug=False,
                   num_devices=N_CORES if collective else 1)
    qt_in = None
    if not collective:
        qt_in = nc.declare_dram_parameter("qtin", [DCW, DCN * B], f16,
                                          isOutput=False)

    ktg_in = nc.declare_dram_parameter("ktg", [DCW, NCH * DCN * CHUNK], f16,
                                       isOutput=False)
    et_in = nc.declare_dram_parameter("et", [DCW, DCN * TOK], f16,
                                      isOutput=False)
    wt_in = nc.declare_dram_parameter("wt", [DCW, sum(KSIZES) * DCN * KN], f16,
                                      isOutput=False)
    bias_in = nc.declare_dram_parameter("bias", [KN, len(KSIZES)], f32,
                                        isOutput=False)
    y_in = nc.declare_dram_parameter("yv", [B, 1], f32, isOutput=False)
    blab_in = nc.declare_dram_parameter("blab", [1, NBLK], f16, isOutput=False)

    pn_out = nc.declare_dram_parameter("pn", [B, 2], f32, isOutput=True)
    ss_out = nc.declare_dram_parameter("ss", [1, B], f32, isOutput=True)

    cc_in = nc.dram_tensor("cc_in", [DCW, DCN * BPC], f16)
    cc_out = nc.dram_tensor("cc_out", [N_CORES, DCW, DCN * BPC], f16,
                            addr_space="Shared")

    woff = {}   # column offset of each kernel size's weights in wt
    off = 0
    for k in KSIZES:
        woff[k] = off
        off += k * DCN * KN

    # mask windows in block units: wide super windows, then fine chunks
    winb = []
    s_acc = 0
    for wsz in WIN_SUP:
        winb.append((s_acc * SUPB, wsz * SUPB))
        s_acc += wsz
    for c in range(FINE):
        winb.append((NSUP * SUPB + c * CPB, CPB))
    nwin = len(winb)
    maxwb = max(wd for _, wd in winb)

    with tile.TileContext(nc) as tc, ExitStack() as ctx:
        singles = ctx.enter_context(tc.tile_pool(name="singles", bufs=1))
        ktp = ctx.enter_context(tc.tile_pool(name="ktp", bufs=ktbufs))
        scp = ctx.enter_context(tc.tile_pool(name="scp", bufs=scbufs))

        # ------------- input DMAs (SP queue; no long waits) ---------------
        # split so the k=3 conv can start as soon as its operands land
        qtall = singles.tile([DCW, DCN * B], f16, tag="qtall")
        et = singles.tile([DCW, DCN * TOK], f16, tag="et")
        nc.sync.dma_start(out=et[:, :TOK], in_=et_in[:, :TOK])
        wsplit = KSIZES[0] * DCN * KN
        wt = singles.tile([DCW, sum(KSIZES) * DCN * KN], f16, tag="wt")
        nc.sync.dma_start(out=wt[:, :wsplit], in_=wt_in[:, :wsplit])
        nc.sync.dma_start(out=et[:, TOK:], in_=et_in[:, TOK:])
        nc.sync.dma_start(out=wt[:, wsplit:], in_=wt_in[:, wsplit:])
        bia = singles.tile([KN, len(KSIZES)], f32, tag="bias")
        nc.sync.dma_start(out=bia, in_=bias_in[:, :])
        if not collective:
            # no collective: queries come straight from DRAM; early in the
            # queue so the stream can start the moment the CNN finishes
            nc.sync.dma_start(out=qtall, in_=qt_in[:, :])
        y0 = singles.tile([B, 1], f32, tag="y0")
        nc.sync.dma_start(out=y0, in_=y_in[:, :])
        # block labels broadcast to all partitions via stride-0 DMA read
        blabB = singles.tile([B, NBLK], f16, tag="blabB")
        nc.sync.dma_start(out=blabB, in_=bass.AP(
            tensor=blab_in.ap().tensor, offset=0, ap=[[0, B], [1, NBLK]]))

        # ---------------- mask prep (overlaps CNN; DVE idle then) ---------
        eq01 = singles.tile([B, NBLK], f16, tag="eq01")
        nc.vector.tensor_scalar(out=eq01[:], in0=blabB[:], scalar1=y0[:],
                                scalar2=None, op0=mybir.AluOpType.is_equal)
        # eqp = +BIG where label==y else -BIG ; eqn = the opposite
        eqp = singles.tile([B, NBLK], f16, tag="eqp")
        nc.vector.tensor_scalar(out=eqp[:], in0=eq01[:], scalar1=2.0 * BIG,
                                scalar2=-BIG, op0=mybir.AluOpType.mult,
                                op1=mybir.AluOpType.add)
        eqn = singles.tile([B, NBLK], f16, tag="eqn")
        nc.vector.tensor_scalar(out=eqn[:], in0=eq01[:], scalar1=-2.0 * BIG,
                                scalar2=BIG, op0=mybir.AluOpType.mult,
                                op1=mybir.AluOpType.add)

        # ---------------- CNN phase ----------------
        feats = {}  # per kernel size: [100, BPC] f32 (this IS a qT d-chunk)
        auxps = ctx.enter_context(tc.tile_pool(name="auxps", bufs=1,
                                               space="PSUM"))
        with tc.tile_pool(name="cnnps", bufs=3, space="PSUM") as cnnps, \
             tc.tile_pool(name="cnnsb", bufs=4) as cnnsb:
            half = BPC // 2
            if warmup:
                # PE p-state warmup: one long zero accumulation group that
                # runs while the et/wt DMAs land, so CNN matmuls start at
                # full clock. The warm PSUM tile doubles as stream super
                # 0's tile, dodging the CNN pools' bank-reuse WAR.
                wz1 = singles.tile([1, B], f16, tag="wz1")
                nc.vector.memset(wz1, 0.0)
                wz2 = singles.tile([1, warmw], f16, tag="wz2")
                nc.vector.memset(wz2, 0.0)
                wps = auxps.tile([B, SW * CHUNK], f32, tag="warm")
                for i in range(warmup):
                    nc.tensor.matmul(wps[:, 0:warmw], wz1[:], wz2[:],
                                     start=(i == 0), stop=(i == warmup - 1))
            for ki, k in enumerate(KSIZES):
                lout = L - k + 1
                fk = singles.tile([KN, BPC], f32, tag=f"feats{k}")
                feats[k] = fk
                pss = [cnnps.tile([KN, half * lout], f32,
                                  name=f"cnnpsum{k}_{h}", tag="cnnpsum")
                       for h in range(2)]
                # dc-outer so the first matmuls only need the first et third;
                # h interleaved so PE stays busy while later thirds land
                for dc in range(DCN):
                    for h in range(2):
                        rhs_full = et[:, dc * TOK:(dc + 1) * TOK].rearrange(
                            "p (b l) -> p b l", l=L)
                        for t in range(k):
                            nc.tensor.matmul(
                                pss[h][:],
                                wt[:, woff[k] + (t * DCN + dc) * KN:
                                   woff[k] + (t * DCN + dc + 1) * KN],
                                rhs_full[:, h * half:(h + 1) * half,
                                         t:t + lout],
                                start=(dc == 0 and t == 0),
                                stop=(dc == DCN - 1 and t == k - 1))
                for h in range(2):
                    # bias + relu (ACT), then maxpool over positions (DVE)
                    rk = cnnsb.tile([KN, half * lout], f32, tag="relu")
                    nc.scalar.activation(rk[:], pss[h][:],
                                         mybir.ActivationFunctionType.Relu,
                                         bias=bia[:, ki:ki + 1], scale=1.0)
                    nc.vector.tensor_reduce(
                        out=fk[:, h * half:(h + 1) * half],
                        in_=rk.rearrange("p (b l) -> p b l", l=lout),
                        axis=mybir.AxisListType.X, op=mybir.AluOpType.max)

            # qT assembly: features in f16, AllGathered across cores. The
            # collective chain rides the ACT queue: everything emitted
            # after it on that queue depends on the AllGather anyway.
            if collective:
                fall = singles.tile([DCW, DCN * BPC], f16, tag="fall")
                for i, k in enumerate(KSIZES):
                    nc.vector.tensor_copy(
                        fall[:, i * BPC:(i + 1) * BPC], feats[k][:, :])
                nc.scalar.dma_start(out=cc_in[:, :], in_=fall[:])
                nc.gpsimd.collective_compute(
                    "AllGather", mybir.AluOpType.bypass,
                    replica_groups=[list(range(N_CORES))],
                    ins=[cc_in[:, :]], outs=[cc_out[:, :, :]])
                # qtall[p, dc*B + core*BPC + i] = cc_out[core, p, dc*BPC + i]
                src = bass.AP(
                    tensor=cc_out.ap().tensor, offset=0,
                    ap=[[DCN * BPC, DCW], [BPC, DCN],
                        [DCW * DCN * BPC, N_CORES], [1, BPC]])
                nc.scalar.dma_start(
                    out=qtall.rearrange("p (dc core i) -> p dc core i",
                                        dc=DCN, core=N_CORES),
                    in_=src)

        # ---------------- memory stream ----------------
        # PE fills two-bank PSUM supers; ACT copies them to f16; DVE turns
        # each super into 16-wide block maxes with a 4-level pairwise-max
        # tree (tensor_tensor gets DVE 2x mode, tensor_reduce does not).
        bmall = singles.tile([B, NBLK], f16, tag="bmall")
        t1 = singles.tile([B, SW * CHUNK // 2], f16, tag="t1")
        t2 = singles.tile([B, SW * CHUNK // 4], f16, tag="t2")
        t3 = singles.tile([B, SW * CHUNK // 8], f16, tag="t3")
        pm = singles.tile([B, maxwb], f16, tag="pm")
        parts_p = singles.tile([B, nwin], f32, tag="parts_p")
        parts_n = singles.tile([B, nwin], f32, tag="parts_n")
        pn = singles.tile([B, 2], f32, tag="pn")

        sizes = _group_sizes(g)
        starts = [sum(sizes[:i]) for i in range(len(sizes))]
        nwid = SW * DCN * CHUNK          # slab columns per super
        mx = mybir.AluOpType.max
        win_after = {}                   # block-end -> window index
        bacc_ = 0
        for wi, (b0, wd) in enumerate(winb):
            win_after[b0 + wd] = wi

        def block_tree(sc_ap, nch_w, bout):
            # pairwise-max tree: [B, nch_w*CHUNK] f16 -> [B, nch_w*CPB]
            nb = nch_w * CPB
            v0 = sc_ap.rearrange("p (nb blk) -> p nb blk", blk=BLK)
            v1 = t1[:, :nb * 8].rearrange("p (nb blk) -> p nb blk", blk=8)
            nc.vector.tensor_tensor(out=v1, in0=v0[:, :, 0:8],
                                    in1=v0[:, :, 8:16], op=mx)
            v2 = t2[:, :nb * 4].rearrange("p (nb blk) -> p nb blk", blk=4)
            nc.vector.tensor_tensor(out=v2, in0=v1[:, :, 0:4],
                                    in1=v1[:, :, 4:8], op=mx)
            v3 = t3[:, :nb * 2].rearrange("p (nb blk) -> p nb blk", blk=2)
            nc.vector.tensor_tensor(out=v3, in0=v2[:, :, 0:2],
                                    in1=v2[:, :, 2:4], op=mx)
            nc.vector.tensor_tensor(out=bout, in0=v3[:, :, 0:1].rearrange(
                "p nb one -> p (nb one)"), in1=v3[:, :, 1:2].rearrange(
                "p nb one -> p (nb one)"), op=mx)

        def fold_windows(bend):
            # fold any window ending at block `bend` into pos/neg partials
            wi = win_after.get(bend)
            if wi is None:
                return
            b0, wd = winb[wi]
            sl = slice(b0, b0 + wd)
            nc.vector.tensor_tensor(out=pm[:, :wd], in0=bmall[:, sl],
                                    in1=eqp[:, sl], op=mybir.AluOpType.min)
            nc.vector.tensor_reduce(out=parts_p[:, wi:wi + 1],
                                    in_=pm[:, :wd],
                                    axis=mybir.AxisListType.X, op=mx)
            nc.vector.tensor_tensor(out=pm[:, :wd], in0=bmall[:, sl],
                                    in1=eqn[:, sl], op=mybir.AluOpType.min)
            nc.vector.tensor_reduce(out=parts_n[:, wi:wi + 1],
                                    in_=pm[:, :wd],
                                    axis=mybir.AxisListType.X, op=mx)

        with tc.tile_pool(name="simps", bufs=2, space="PSUM") as simps, \
             tc.tile_pool(name="ssp", bufs=1, space="PSUM") as ssp:
            # feature norms for ALL batch rows, straight from the gathered
            # qT (gated only on qtall): the f16 feature quantization is
            # baked into the sims, so this is the consistent normalizer.
            ones = singles.tile([DCW, 1], f32, tag="ones")
            nc.vector.memset(ones, 1.0)
            sq2 = singles.tile([DCW, DCN * B], f32, tag="sq2")
            nc.vector.tensor_mul(sq2[:], qtall[:], qtall[:])
            ssps = ssp.tile([1, B], f32, tag="ssps")
            for dc in range(DCN):
                nc.tensor.matmul(ssps[:], ones[:],
                                 sq2[:, dc * B:(dc + 1) * B],
                                 start=(dc == 0), stop=(dc == DCN - 1))
            ss_sb = singles.tile([1, B], f32, tag="ss_sb")
            nc.vector.tensor_copy(ss_sb[:], ssps[:])
            nc.scalar.dma_start(out=ss_out[:, :], in_=ss_sb[:])

            # wide region: super-granular groups
            for gi, (s0, gsz) in enumerate(zip(starts, sizes)):
                kt = ktp.tile([DCW, g * nwid], f16, tag="kt")
                gw = gsz * nwid
                nc.sync.dma_start(
                    out=kt[:, :gw], in_=ktg_in[:, s0 * nwid:s0 * nwid + gw])
                for sl in range(gsz):
                    s = s0 + sl
                    if s == 0 and warmup:
                        ps = wps      # warm tile: no CNN-pool bank WAR
                    else:
                        ps = simps.tile([B, SW * CHUNK], f32, name="simpsum",
                                        tag="simpsum")
                    for sub in range(SW):
                        for dc in range(DCN):
                            nc.tensor.matmul(
                                ps[:, sub * CHUNK:(sub + 1) * CHUNK],
                                qtall[:, dc * B:(dc + 1) * B],
                                kt[:, ((sl * SW + sub) * DCN + dc) * CHUNK:
                                   ((sl * SW + sub) * DCN + dc + 1) * CHUNK],
                                start=(dc == 0), stop=(dc == DCN - 1))
                    sc = scp.tile([B, SW * CHUNK], f16, name="scw",
                                  tag="simf16")
                    nc.scalar.copy(sc[:], ps[:])
                    block_tree(sc[:], SW, bmall[:, s * SUPB:(s + 1) * SUPB])
                    fold_windows((s + 1) * SUPB)
            # fine tail: chunk-granular for a short closing chain
            for i in range(FINE):
                c = NCH - FINE + i
                ktc = ktp.tile([DCW, g * nwid], f16, name="ktc", tag="kt")
                nc.sync.dma_start(
                    out=ktc[:, :DCN * CHUNK],
                    in_=ktg_in[:, c * DCN * CHUNK:(c + 1) * DCN * CHUNK])
                ps = simps.tile([B, SW * CHUNK], f32, name="simpsumf",
                                tag="simpsum")
                for dc in range(DCN):
                    nc.tensor.matmul(
                        ps[:, 0:CHUNK], qtall[:, dc * B:(dc + 1) * B],
                        ktc[:, dc * CHUNK:(dc + 1) * CHUNK],
                        start=(dc == 0), stop=(dc == DCN - 1))
                sc = scp.tile([B, SW * CHUNK], f16, name="scf", tag="simf16")
                nc.scalar.copy(sc[:, 0:CHUNK], ps[:, 0:CHUNK])
                block_tree(sc[:, 0:CHUNK], 1,
                           bmall[:, c * CPB:(c + 1) * CPB])
                fold_windows((c + 1) * CPB)

        # final: max over window partials -> pos/neg, ship each ASAP
        nc.vector.tensor_reduce(out=pn[:, 0:1], in_=parts_p[:],
                                axis=mybir.AxisListType.X, op=mx)
        nc.scalar.dma_start(out=pn_out[:, 0:1], in_=pn[:, 0:1])
        nc.vector.tensor_reduce(out=pn[:, 1:2], in_=parts_n[:],
                                axis=mybir.AxisListType.X, op=mx)
        nc.sync.dma_start(out=pn_out[:, 1:2], in_=pn[:, 1:2])

    nc.compile()
    return nc


def _prep(x, y, embed, conv_w3, conv_b3, conv_w4, conv_b4, conv_w5, conv_b5,
          mem_keys, mem_values):
    """Host-side sharding/packing. Returns per-core input maps + combine data."""
    x = np.asarray(x)
    y64 = np.asarray(y).astype(np.int64)
    mv = np.asarray(mem_values).astype(np.int64)
    mk = np.asarray(mem_keys, dtype=np.float32)

    # --- label-sorted, block-pure padded permutation of the memory bank ---
    order = np.argsort(mv, kind="stable")
    cnt = np.bincount(mv, minlength=C)
    assert cnt.min() > 0, "kernel assumes every class present in memory"
    cstarts = np.zeros(C + 1, np.int64)
    cstarts[1:] = np.cumsum(cnt)
    parts = []
    for c in range(C):
        grp = order[cstarts[c]:cstarts[c + 1]]
        padn = (-len(grp)) % BLK
        if padn:
            grp = np.concatenate([grp, np.repeat(grp[0], padn)])
        parts.append(grp)
    perm = np.concatenate(parts)
    assert len(perm) <= CAP, f"padded size {len(perm)} exceeds CAP {CAP}"
    # tail pad duplicates the LAST class (keeps every class contiguous and
    # block-pure; the duplicates are real keys so maxes are exact)
    perm = np.concatenate([perm, np.repeat(parts[-1][0], CAP - len(perm))])
    labP = mv[perm]
    blab = labP[::BLK].astype(np.float16)          # [CAP // BLK]
    keysP = mk.astype(np.float16)[perm]            # cast before gather: half the traffic

    # --- embedding lookup (host gather; device gets ready eT slabs) ---
    emb16 = np.asarray(embed, dtype=np.float32).astype(np.float16)
    e = emb16[x]                                    # [B, L, 300]

    # --- conv weights packed into one tensor:
    #     wt[p, woff_k + (t*DCN+dc)*KN + kn] = w_k[kn, dc*100+p, t]
    wparts = []
    for k, w_ in ((3, conv_w3), (4, conv_w4), (5, conv_w5)):
        w_ = np.asarray(w_, dtype=np.float32)       # [KN, D, k]
        a = w_.reshape(KN, DCN, DCW, k).transpose(2, 3, 1, 0)  # [p, t, dc, kn]
        wparts.append(a.reshape(DCW, k * DCN * KN))
    wt = np.ascontiguousarray(np.concatenate(wparts, axis=1)).astype(np.float16)

    biases = np.zeros((KN, len(KSIZES)), np.float32)
    for i, b_ in enumerate((conv_b3, conv_b4, conv_b5)):
        biases[:, i] = np.asarray(b_, dtype=np.float32)

    yv = y64.astype(np.float32).reshape(B, 1)

    in_maps = []
    for c in range(N_CORES):
        # key slab packed chunk-major: ktg[p, (j*DCN+dc)*CHUNK + cc]
        #   = keysP[c*W + j*CHUNK + cc, dc*DCW + p]
        slab = keysP[c * W:(c + 1) * W]             # [W, 300]
        ktg = np.ascontiguousarray(
            slab.reshape(NCH, CHUNK, DCN, DCW)
            .transpose(3, 0, 2, 1)                  # [p, j, dc, cc]
            .reshape(DCW, NCH * DCN * CHUNK))
        # eT: et[p, dc*TOK + b*L + l] = e[c*BPC + b, l, dc*100 + p]
        ec = e[c * BPC:(c + 1) * BPC]               # [BPC, L, 300]
        et = np.ascontiguousarray(
            ec.reshape(BPC, L, DCN, DCW)
            .transpose(3, 2, 0, 1)                  # [p, dc, b, l]
            .reshape(DCW, DCN * TOK))
        m = {
            "ktg": ktg,
            "et": et,
            "wt": wt,
            "bias": biases,
            "yv": yv,
            "blab": np.ascontiguousarray(
                blab[c * NBLK:(c + 1) * NBLK]).reshape(1, NBLK),
        }
        in_maps.append(m)
    return in_maps, y64


def _combine(results, y64):
    pos = np.max([r["pn"][:, 0] for r in results], axis=0)
    neg = np.max([r["pn"][:, 1] for r in results], axis=0)
    ss = results[0]["ss"].reshape(B)    # every core computes all norms
    rn = 1.0 / np.maximum(np.sqrt(ss), 1e-12)
    sp = pos * rn
    sn = neg * rn
    loss = np.float32(np.mean(np.maximum(sn - sp + MARGIN, 0.0)))
    acc = np.float32(np.mean((sp > sn).astype(np.float32)))
    return loss, acc


def kernel(**inputs):
    global _CACHED_NC
    in_maps, y64 = _prep(**inputs)
    if _CACHED_NC is None:
        _CACHED_NC = build()
    res = run_bass_kernel_spmd(_CACHED_NC, in_maps,
                               core_ids=list(range(N_CORES)))
    return _combine(res.results, y64)


# revision 61
# speedup vs baseline: 1.9387x; 1.0252x over previous
"""Trainium2 Bass kernel for nn_CNN_Mem (CNN text encoder + cosine memory lookup).

Strategy (8 NeuronCores, SPMD):
  - Memory bank sharded along mem_size: host label-sorts mem_keys so every
    16-column block holds a single label (per-class groups padded by
    duplicating a real key of the same class; tail padded with the last
    class), casts to fp16, transposes to [300, M/8] slabs per core, and
    packs the three 100-row d-chunks chunk-major so each stream group is
    ONE contiguous DMA.
  - Each core: CNN for its 16 batch rows (embedding rows gathered host-side,
    convs as PSUM-accumulated matmuls over shifted APs, relu+bias on ACT,
    maxpool on DVE) -> feature chunks [100, 16] per kernel size = the
    d-chunks of q^T. AllGather across the 8 cores -> qT [100, 128] per
    d-chunk (single strided readback DMA).
  - Stream the packed key slab through the PE in [128, 1024] two-bank PSUM
    super-chunks (6 accumulated fp16 matmuls); ACT copies PSUM->SBUF f16;
    DVE computes per-block (16-wide) maxes with a 4-level pairwise-max
    tree (tensor_tensor runs in 2x mode, unlike tensor_reduce) -> bmall.
  - Masked maxes at block level: block labels are DMA-broadcast across
    partitions, compared against y (is_equal) and turned into +-BIG
    sentinels; per window, min(bmall, mask) + max-reduce give per-window
    sim_pos / sim_neg partials; a final max over windows produces pos/neg.
    Windows narrow toward the end of the stream (the last two chunks run
    chunk-granular) so the closing chain after the final matmul is short.
  - Feature norms come from the gathered qT itself (ones-matmul of its
    square), so the PE queue never waits on the CNN maxpool chain.
  - Host combines: max over cores, divide by feature norms (maxes commute
    with the positive per-row normalization), then margin loss + accuracy
    (sp > sn is equivalent to the argmax form for distinct maxima).

Performance notes (TimelineSim + HW-validated instruction set):
  - fp16 keys halve streamed bytes; DMA floor ~57us/core at the modeled
    360 B/ns aggregate DMA bandwidth. fp32 margin analysis: per-row
    |sim_pos - sim_neg| >= 0.034 vs ~1e-4 fp16-induced error.
  - One DMA per stream group (packed layout) keeps the shared descriptor
    generator (HWDGE, ~0.6us per DMA) off the critical path.
  - PE p-state warmup (one long zero accumulation group) covers the input
    DMA window so CNN and stream matmuls all dispatch at full clock; its
    PSUM tile doubles as stream super 0's, dodging the CNN-pool bank WAR.
  - Queue assignment: SP carries the input stream, ACT the collective
    chain (harmless: everything behind it needs the AllGather anyway) and
    outputs. Only HW-proven instructions are used (tensor_mask_reduce /
    tensor_tensor_reduce / gpsimd ALU ops all fail neuronxcc on TRN2).
"""
import numpy as np
from contextlib import ExitStack

import concourse.bass as bass
import concourse.tile as tile
from concourse import bacc, mybir
from concourse.bass_utils import run_bass_kernel_spmd

# ---- problem dims (hardcoded; harness passes matching inputs) ----
B, L = 128, 64
V, D = 25000, 300
C = 1000
KN = 100
KSIZES = (3, 4, 5)
M, KEY = 262144, 300
MARGIN = 0.1

N_CORES = 8
BPC = B // N_CORES          # batch rows per core
TOK = BPC * L               # tokens per core
DCN = 3                     # d-chunks of 100
DCW = 100                   # d-chunk width
CHUNK = 512                 # sim columns per PSUM bank
BLK = 16                    # label-pure block width
NCH = 66                    # chunks per core
W = NCH * CHUNK             # slab columns per core (33792)
CAP = N_CORES * W           # padded memory size (270336)
NBLK = W // BLK             # blocks per core (2112)
CPB = CHUNK // BLK          # blocks per chunk (32)
BIG = 16.0                  # mask sentinel; |sims_u| <= ~8 so +-16 acts as +-inf

SW = 2                      # chunks per super-chunk (PSUM/ACT granularity)
FINE = 2                    # trailing chunks run chunk-granular (short tail)
NSUP = (NCH - FINE) // SW   # wide supers per core (32)
SUPB = SW * CPB             # blocks per super (64)
# masked-min partials folded every FS supers (fewer DVE ops) with batched
# segmented max-reduces; batches shrink toward the end for a short tail
FS = 2                                        # supers per fold segment
NSEG = NSUP // FS + 1                         # segments (16 wide + fine)
RED_BATCH = (2, 2, 2, 2, 2, 2, 2, 1, 1, 1)    # in segments, sum = NSEG

f32 = mybir.dt.float32
f16 = mybir.dt.float16

_CACHED_NC = None


def _group_sizes(g, taper=(1, 1, 1)):
    left = NSUP - sum(taper)
    sizes = []
    while left > 0:
        s = min(g, left)
        sizes.append(s)
        left -= s
    return sizes + list(taper)


def build(collective=True, g=2, ktbufs=6, scbufs=6, warmup=15, warmw=256):
    nc = bacc.Bacc("TRN2", target_bir_lowering=False, debug=False,
                   num_devices=N_CORES if collective else 1)
    qt_in = None
    if not collective:
        qt_in = nc.declare_dram_parameter("qtin", [DCW, DCN * B], f16,
                                          isOutput=False)

    ktg_in = nc.declare_dram_parameter("ktg", [DCW, NCH * DCN * CHUNK], f16,
                                       isOutput=False)
    et_in = nc.declare_dram_parameter("et", [DCW, DCN * TOK], f16,
                                      isOutput=False)
    wt_in = nc.declare_dram_parameter("wt", [DCW, sum(KSIZES) * DCN * KN], f16,
                                      isOutput=False)
    bias_in = nc.declare_dram_parameter("bias", [KN, len(KSIZES)], f32,
                                        isOutput=False)
    y_in = nc.declare_dram_parameter("yv", [B, 1], f32, isOutput=False)
    blab_in = nc.declare_dram_parameter("blab", [1, NBLK], f16, isOutput=False)

    pn_out = nc.declare_dram_parameter("pn", [B, 2], f32, isOutput=True)
    ss_out = nc.declare_dram_parameter("ss", [1, B], f32, isOutput=True)

    cc_in = nc.dram_tensor("cc_in", [DCW, DCN * BPC], f16)
    cc_out = nc.dram_tensor("cc_out", [N_CORES, DCW, DCN * BPC], f16,
                            addr_space="Shared")

    woff = {}   # column offset of each kernel size's weights in wt
    off = 0
    for k in KSIZES:
        woff[k] = off
        off += k * DCN * KN

    # reduce-batch boundaries: after these segment indices (fine = last)
    nseg = NSEG
    segb = FS * SUPB                             # blocks per wide segment
    assert sum(RED_BATCH) == nseg
    batch_after = {}
    acc = 0
    for bk in RED_BATCH:
        acc += bk
        batch_after[acc - 1] = (acc - bk, bk)    # last seg -> (start, size)

    with tile.TileContext(nc) as tc, ExitStack() as ctx:
        singles = ctx.enter_context(tc.tile_pool(name="singles", bufs=1))
        ktp = ctx.enter_context(tc.tile_pool(name="ktp", bufs=ktbufs))
        scp = ctx.enter_context(tc.tile_pool(name="scp", bufs=scbufs))

        # ------------- input DMAs (SP queue; no long waits) ---------------
        # split so the k=3 conv can start as soon as its operands land
        qtall = singles.tile([DCW, DCN * B], f16, tag="qtall")
        et = singles.tile([DCW, DCN * TOK], f16, tag="et")
        nc.sync.dma_start(out=et[:, :TOK], in_=et_in[:, :TOK])
        wsplit = KSIZES[0] * DCN * KN
        wt = singles.tile([DCW, sum(KSIZES) * DCN * KN], f16, tag="wt")
        nc.sync.dma_start(out=wt[:, :wsplit], in_=wt_in[:, :wsplit])
        nc.sync.dma_start(out=et[:, TOK:], in_=et_in[:, TOK:])
        nc.sync.dma_start(out=wt[:, wsplit:], in_=wt_in[:, wsplit:])
        bia = singles.tile([KN, len(KSIZES)], f32, tag="bias")
        nc.sync.dma_start(out=bia, in_=bias_in[:, :])
        if not collective:
            # no collective: queries come straight from DRAM; early in the
            # queue so the stream can start the moment the CNN finishes
            nc.sync.dma_start(out=qtall, in_=qt_in[:, :])
        y0 = singles.tile([B, 1], f32, tag="y0")
        nc.sync.dma_start(out=y0, in_=y_in[:, :])
        # block labels broadcast to all partitions via stride-0 DMA read
        blabB = singles.tile([B, NBLK], f16, tag="blabB")
        nc.sync.dma_start(out=blabB, in_=bass.AP(
            tensor=blab_in.ap().tensor, offset=0, ap=[[0, B], [1, NBLK]]))

        # ---------------- mask prep (overlaps CNN; DVE idle then) ---------
        eq01 = singles.tile([B, NBLK], f16, tag="eq01")
        nc.vector.tensor_scalar(out=eq01[:], in0=blabB[:], scalar1=y0[:],
                                scalar2=None, op0=mybir.AluOpType.is_equal)
        # eqp = +BIG where label==y else -BIG ; eqn = the opposite
        eqp = singles.tile([B, NBLK], f16, tag="eqp")
        nc.vector.tensor_scalar(out=eqp[:], in0=eq01[:], scalar1=2.0 * BIG,
                                scalar2=-BIG, op0=mybir.AluOpType.mult,
                                op1=mybir.AluOpType.add)
        eqn = singles.tile([B, NBLK], f16, tag="eqn")
        nc.vector.tensor_scalar(out=eqn[:], in0=eq01[:], scalar1=-2.0 * BIG,
                                scalar2=BIG, op0=mybir.AluOpType.mult,
                                op1=mybir.AluOpType.add)

        # ---------------- CNN phase ----------------
        feats = {}  # per kernel size: [100, BPC] f32 (this IS a qT d-chunk)
        auxps = ctx.enter_context(tc.tile_pool(name="auxps", bufs=1,
                                               space="PSUM"))
        with tc.tile_pool(name="cnnps", bufs=3, space="PSUM") as cnnps, \
             tc.tile_pool(name="cnnsb", bufs=4) as cnnsb:
            half = BPC // 2
            if warmup:
                # PE p-state warmup: one long zero accumulation group that
                # runs while the et/wt DMAs land, so CNN matmuls start at
                # full clock. The warm PSUM tile doubles as stream super
                # 0's tile, dodging the CNN pools' bank-reuse WAR.
                wz1 = singles.tile([1, B], f16, tag="wz1")
                nc.vector.memset(wz1, 0.0)
                wz2 = singles.tile([1, warmw], f16, tag="wz2")
                nc.vector.memset(wz2, 0.0)
                wps = auxps.tile([B, SW * CHUNK], f32, tag="warm")
                for i in range(warmup):
                    nc.tensor.matmul(wps[:, 0:warmw], wz1[:], wz2[:],
                                     start=(i == 0), stop=(i == warmup - 1))
            for ki, k in enumerate(KSIZES):
                lout = L - k + 1
                fk = singles.tile([KN, BPC], f32, tag=f"feats{k}")
                feats[k] = fk
                pss = [cnnps.tile([KN, half * lout], f32,
                                  name=f"cnnpsum{k}_{h}", tag="cnnpsum")
                       for h in range(2)]
                # dc-outer so the first matmuls only need the first et third;
                # h interleaved so PE stays busy while later thirds land
                for dc in range(DCN):
                    for h in range(2):
                        rhs_full = et[:, dc * TOK:(dc + 1) * TOK].rearrange(
                            "p (b l) -> p b l", l=L)
                        for t in range(k):
                            nc.tensor.matmul(
                                pss[h][:],
                                wt[:, woff[k] + (t * DCN + dc) * KN:
                                   woff[k] + (t * DCN + dc + 1) * KN],
                                rhs_full[:, h * half:(h + 1) * half,
                                         t:t + lout],
                                start=(dc == 0 and t == 0),
                                stop=(dc == DCN - 1 and t == k - 1))
                for h in range(2):
                    # bias + relu (ACT), then maxpool over positions (DVE)
                    rk = cnnsb.tile([KN, half * lout], f32, tag="relu")
                    nc.scalar.activation(rk[:], pss[h][:],
                                         mybir.ActivationFunctionType.Relu,
                                         bias=bia[:, ki:ki + 1], scale=1.0)
                    nc.vector.tensor_reduce(
                        out=fk[:, h * half:(h + 1) * half],
                        in_=rk.rearrange("p (b l) -> p b l", l=lout),
                        axis=mybir.AxisListType.X, op=mybir.AluOpType.max)

            # qT assembly: features in f16, AllGathered across cores. The
            # collective chain rides the ACT queue: everything emitted
            # after it on that queue depends on the AllGather anyway.
            if collective:
                fall = singles.tile([DCW, DCN * BPC], f16, tag="fall")
                for i, k in enumerate(KSIZES):
                    nc.vector.tensor_copy(
                        fall[:, i * BPC:(i + 1) * BPC], feats[k][:, :])
                nc.scalar.dma_start(out=cc_in[:, :], in_=fall[:])
                nc.gpsimd.collective_compute(
                    "AllGather", mybir.AluOpType.bypass,
                    replica_groups=[list(range(N_CORES))],
                    ins=[cc_in[:, :]], outs=[cc_out[:, :, :]])
                # qtall[p, dc*B + core*BPC + i] = cc_out[core, p, dc*BPC + i]
                src = bass.AP(
                    tensor=cc_out.ap().tensor, offset=0,
                    ap=[[DCN * BPC, DCW], [BPC, DCN],
                        [DCW * DCN * BPC, N_CORES], [1, BPC]])
                nc.scalar.dma_start(
                    out=qtall.rearrange("p (dc core i) -> p dc core i",
                                        dc=DCN, core=N_CORES),
                    in_=src)

        # ---------------- memory stream ----------------
        # PE fills two-bank PSUM supers; ACT copies them to f16; DVE turns
        # each super into 16-wide block maxes with a 4-level pairwise-max
        # tree (tensor_tensor gets DVE 2x mode, tensor_reduce does not).
        bmall = singles.tile([B, NBLK], f16, tag="bmall")
        t1 = singles.tile([B, SW * CHUNK // 2], f16, tag="t1")
        t2 = singles.tile([B, SW * CHUNK // 4], f16, tag="t2")
        t3 = singles.tile([B, SW * CHUNK // 8], f16, tag="t3")
        pmp = singles.tile([B, NBLK], f16, tag="pmp")
        pmn = singles.tile([B, NBLK], f16, tag="pmn")
        parts_p = singles.tile([B, nseg], f32, tag="parts_p")
        parts_n = singles.tile([B, nseg], f32, tag="parts_n")
        pn = singles.tile([B, 2], f32, tag="pn")

        sizes = _group_sizes(g)
        starts = [sum(sizes[:i]) for i in range(len(sizes))]
        nwid = SW * DCN * CHUNK          # slab columns per super
        mx = mybir.AluOpType.max

        def block_tree(sc_ap, nb, bout):
            # pairwise-max tree: [B, nb*BLK] f16 -> [B, nb] block maxes
            v0 = sc_ap.rearrange("p (nb blk) -> p nb blk", blk=BLK)
            v1 = t1[:, :nb * 8].rearrange("p (nb blk) -> p nb blk", blk=8)
            nc.vector.tensor_tensor(out=v1, in0=v0[:, :, 0:8],
                                    in1=v0[:, :, 8:16], op=mx)
            v2 = t2[:, :nb * 4].rearrange("p (nb blk) -> p nb blk", blk=4)
            nc.vector.tensor_tensor(out=v2, in0=v1[:, :, 0:4],
                                    in1=v1[:, :, 4:8], op=mx)
            v3 = t3[:, :nb * 2].rearrange("p (nb blk) -> p nb blk", blk=2)
            nc.vector.tensor_tensor(out=v3, in0=v2[:, :, 0:2],
                                    in1=v2[:, :, 2:4], op=mx)
            nc.vector.tensor_tensor(out=bout, in0=v3[:, :, 0:1].rearrange(
                "p nb one -> p (nb one)"), in1=v3[:, :, 1:2].rearrange(
                "p nb one -> p (nb one)"), op=mx)

        def fold_seg(si, blocks):
            # masked segment mins; batched segmented max-reduce over
            # equal-width wide segments (the fine segment reduces alone)
            b0b, b1b = si * segb, si * segb + blocks
            sl = slice(b0b, b1b)
            nc.vector.tensor_tensor(out=pmp[:, sl], in0=bmall[:, sl],
                                    in1=eqp[:, sl], op=mybir.AluOpType.min)
            nc.vector.tensor_tensor(out=pmn[:, sl], in0=bmall[:, sl],
                                    in1=eqn[:, sl], op=mybir.AluOpType.min)
            ba = batch_after.get(si)
            if ba is None:
                return
            s0_, bk = ba
            for pm_, parts in ((pmp, parts_p), (pmn, parts_n)):
                if si == nseg - 1:      # fine segment: irregular width
                    nc.vector.tensor_reduce(
                        out=parts[:, si:si + 1], in_=pm_[:, sl],
                        axis=mybir.AxisListType.X, op=mx)
                    continue
                nc.vector.tensor_reduce(
                    out=parts[:, s0_:s0_ + bk],
                    in_=pm_[:, s0_ * segb:(s0_ + bk) * segb].rearrange(
                        "p (k s) -> p k s", s=segb),
                    axis=mybir.AxisListType.X, op=mx)

        with tc.tile_pool(name="simps", bufs=3, space="PSUM") as simps:
            # feature norms for ALL batch rows, straight from the gathered
            # qT (gated only on qtall): the f16 feature quantization is
            # baked into the sims, so this is the consistent normalizer.
            ones = singles.tile([DCW, 1], f32, tag="ones")
            nc.vector.memset(ones, 1.0)
            sq2 = singles.tile([DCW, DCN * B], f32, tag="sq2")
            nc.vector.tensor_mul(sq2[:], qtall[:], qtall[:])
            # reuses the warm bank (WAR on super 0's ACT copy — fine, late)
            ssps = auxps.tile([1, B], f32, name="ssps", tag="warm")
            for dc in range(DCN):
                nc.tensor.matmul(ssps[:], ones[:],
                                 sq2[:, dc * B:(dc + 1) * B],
                                 start=(dc == 0), stop=(dc == DCN - 1))
            ss_sb = singles.tile([1, B], f32, tag="ss_sb")
            nc.vector.tensor_copy(ss_sb[:], ssps[:])
            nc.scalar.dma_start(out=ss_out[:, :], in_=ss_sb[:])

            # wide region: super-granular groups
            for gi, (s0, gsz) in enumerate(zip(starts, sizes)):
                kt = ktp.tile([DCW, g * nwid], f16, tag="kt")
                gw = gsz * nwid
                nc.sync.dma_start(
                    out=kt[:, :gw], in_=ktg_in[:, s0 * nwid:s0 * nwid + gw])
                for sl in range(gsz):
                    s = s0 + sl
                    if s == 0 and warmup:
                        ps = wps      # warm tile: no CNN-pool bank WAR
                    else:
                        ps = simps.tile([B, SW * CHUNK], f32, name="simpsum",
                                        tag="simpsum")
                    for sub in range(SW):
                        for dc in range(DCN):
                            nc.tensor.matmul(
                                ps[:, sub * CHUNK:(sub + 1) * CHUNK],
                                qtall[:, dc * B:(dc + 1) * B],
                                kt[:, ((sl * SW + sub) * DCN + dc) * CHUNK:
                                   ((sl * SW + sub) * DCN + dc + 1) * CHUNK],
                                start=(dc == 0), stop=(dc == DCN - 1))
                    sc = scp.tile([B, SW * CHUNK], f16, name="scw",
                                  tag="simf16")
                    nc.scalar.copy(sc[:], ps[:])
                    block_tree(sc[:], SUPB, bmall[:, s * SUPB:(s + 1) * SUPB])
                    if (s + 1) % FS == 0:
                        fold_seg(s // FS, segb)
            # fine tail: the last two chunks form a pseudo-super fed by two
            # chunk-granular DMAs, so the closing DMA->fold chain is short
            psf = auxps.tile([B, SW * CHUNK], f32, name="psf", tag="warm")
            scf = scp.tile([B, SW * CHUNK], f16, name="scf", tag="simf16")
            for i in range(FINE):
                c = NCH - FINE + i
                ktc = ktp.tile([DCW, g * nwid], f16, name="ktc", tag="kt")
                nc.sync.dma_start(
                    out=ktc[:, :DCN * CHUNK],
                    in_=ktg_in[:, c * DCN * CHUNK:(c + 1) * DCN * CHUNK])
                for dc in range(DCN):
                    nc.tensor.matmul(
                        psf[:, i * CHUNK:(i + 1) * CHUNK],
                        qtall[:, dc * B:(dc + 1) * B],
                        ktc[:, dc * CHUNK:(dc + 1) * CHUNK],
                        start=(dc == 0), stop=(dc == DCN - 1))
                nc.scalar.copy(scf[:, i * CHUNK:(i + 1) * CHUNK],
                               psf[:, i * CHUNK:(i + 1) * CHUNK])
            block_tree(scf[:], SUPB, bmall[:, NSUP * SUPB:])
            fold_seg(nseg - 1, FINE * CPB)

        # final: max over segment partials -> pos/neg, ship each ASAP
        nc.vector.tensor_reduce(out=pn[:, 0:1], in_=parts_p[:],
                                axis=mybir.AxisListType.X, op=mx)
        nc.scalar.dma_start(out=pn_out[:, 0:1], in_=pn[:, 0:1])
        nc.vector.tensor_reduce(out=pn[:, 1:2], in_=parts_n[:],
                                axis=mybir.AxisListType.X, op=mx)
        nc.sync.dma_start(out=pn_out[:, 1:2], in_=pn[:, 1:2])

    nc.compile()
    return nc


def _prep(x, y, embed, conv_w3, conv_b3, conv_w4, conv_b4, conv_w5, conv_b5,
          mem_keys, mem_values):
    """Host-side sharding/packing. Returns per-core input maps + combine data."""
    x = np.asarray(x)
    y64 = np.asarray(y).astype(np.int64)
    mv = np.asarray(mem_values).astype(np.int64)
    mk = np.asarray(mem_keys, dtype=np.float32)

    # --- label-sorted, block-pure padded permutation of the memory bank ---
    order = np.argsort(mv, kind="stable")
    cnt = np.bincount(mv, minlength=C)
    assert cnt.min() > 0, "kernel assumes every class present in memory"
    cstarts = np.zeros(C + 1, np.int64)
    cstarts[1:] = np.cumsum(cnt)
    parts = []
    for c in range(C):
        grp = order[cstarts[c]:cstarts[c + 1]]
        padn = (-len(grp)) % BLK
        if padn:
            grp = np.concatenate([grp, np.repeat(grp[0], padn)])
        parts.append(grp)
    perm = np.concatenate(parts)
    assert len(perm) <= CAP, f"padded size {len(perm)} exceeds CAP {CAP}"
    # tail pad duplicates the LAST class (keeps every class contiguous and
    # block-pure; the duplicates are real keys so maxes are exact)
    perm = np.concatenate([perm, np.repeat(parts[-1][0], CAP - len(perm))])
    labP = mv[perm]
    blab = labP[::BLK].astype(np.float16)          # [CAP // BLK]
    keysP = mk.astype(np.float16)[perm]            # cast before gather: half the traffic

    # --- embedding lookup (host gather; device gets ready eT slabs) ---
    emb16 = np.asarray(embed, dtype=np.float32).astype(np.float16)
    e = emb16[x]                                    # [B, L, 300]

    # --- conv weights packed into one tensor:
    #     wt[p, woff_k + (t*DCN+dc)*KN + kn] = w_k[kn, dc*100+p, t]
    wparts = []
    for k, w_ in ((3, conv_w3), (4, conv_w4), (5, conv_w5)):
        w_ = np.asarray(w_, dtype=np.float32)       # [KN, D, k]
        a = w_.reshape(KN, DCN, DCW, k).transpose(2, 3, 1, 0)  # [p, t, dc, kn]
        wparts.append(a.reshape(DCW, k * DCN * KN))
    wt = np.ascontiguousarray(np.concatenate(wparts, axis=1)).astype(np.float16)

    biases = np.zeros((KN, len(KSIZES)), np.float32)
    for i, b_ in enumerate((conv_b3, conv_b4, conv_b5)):
        biases[:, i] = np.asarray(b_, dtype=np.float32)

    yv = y64.astype(np.float32).reshape(B, 1)

    in_maps = []
    for c in range(N_CORES):
        # key slab packed chunk-major: ktg[p, (j*DCN+dc)*CHUNK + cc]
        #   = keysP[c*W + j*CHUNK + cc, dc*DCW + p]
        slab = keysP[c * W:(c + 1) * W]             # [W, 300]
        ktg = np.ascontiguousarray(
            slab.reshape(NCH, CHUNK, DCN, DCW)
            .transpose(3, 0, 2, 1)                  # [p, j, dc, cc]
            .reshape(DCW, NCH * DCN * CHUNK))
        # eT: et[p, dc*TOK + b*L + l] = e[c*BPC + b, l, dc*100 + p]
        ec = e[c * BPC:(c + 1) * BPC]               # [BPC, L, 300]
        et = np.ascontiguousarray(
            ec.reshape(BPC, L, DCN, DCW)
            .transpose(3, 2, 0, 1)                  # [p, dc, b, l]
            .reshape(DCW, DCN * TOK))
        m = {
            "ktg": ktg,
            "et": et,
            "wt": wt,
            "bias": biases,
            "yv": yv,
            "blab": np.ascontiguousarray(
                blab[c * NBLK:(c + 1) * NBLK]).reshape(1, NBLK),
        }
        in_maps.append(m)
    return in_maps, y64


def _combine(results, y64):
    pos = np.max([r["pn"][:, 0] for r in results], axis=0)
    neg = np.max([r["pn"][:, 1] for r in results], axis=0)
    ss = results[0]["ss"].reshape(B)    # every core computes all norms
    rn = 1.0 / np.maximum(np.sqrt(ss), 1e-12)
    sp = pos * rn
    sn = neg * rn
    loss = np.float32(np.mean(np.maximum(sn - sp + MARGIN, 0.0)))
    acc = np.float32(np.mean((sp > sn).astype(np.float32)))
    return loss, acc


def kernel(**inputs):
    global _CACHED_NC
    in_maps, y64 = _prep(**inputs)
    if _CACHED_NC is None:
        _CACHED_NC = build()
    res = run_bass_kernel_spmd(_CACHED_NC, in_maps,
                               core_ids=list(range(N_CORES)))
    return _combine(res.results, y64)


# revision 64
# speedup vs baseline: 1.9511x; 1.0064x over previous
"""Trainium2 Bass kernel for nn_CNN_Mem (CNN text encoder + cosine memory lookup).

Strategy (8 NeuronCores, SPMD):
  - Memory bank sharded along mem_size: host label-sorts mem_keys so every
    16-column block holds a single label (per-class groups padded by
    duplicating a real key of the same class; tail padded with the last
    class), casts to fp16, transposes to [300, M/8] slabs per core, and
    packs the three 100-row d-chunks chunk-major so each stream group is
    ONE contiguous DMA.
  - Each core: CNN for its 16 batch rows (embedding rows gathered host-side,
    convs as PSUM-accumulated matmuls over shifted APs, relu+bias on ACT,
    maxpool on DVE) -> feature chunks [100, 16] per kernel size = the
    d-chunks of q^T. AllGather across the 8 cores -> qT [100, 128] per
    d-chunk (single strided readback DMA).
  - Stream the packed key slab through the PE in [128, 1024] two-bank PSUM
    super-chunks (6 accumulated fp16 matmuls); ACT copies PSUM->SBUF f16;
    DVE computes per-block (16-wide) maxes with a 4-level pairwise-max
    tree (tensor_tensor runs in 2x mode, unlike tensor_reduce) -> bmall.
  - Masked maxes at block level: block labels are DMA-broadcast across
    partitions, compared against y (is_equal) and turned into +-BIG
    sentinels; per window, min(bmall, mask) + max-reduce give per-window
    sim_pos / sim_neg partials; a final max over windows produces pos/neg.
    Windows narrow toward the end of the stream (the last two chunks run
    chunk-granular) so the closing chain after the final matmul is short.
  - Feature norms come from the gathered qT itself (ones-matmul of its
    square), so the PE queue never waits on the CNN maxpool chain.
  - Host combines: max over cores, divide by feature norms (maxes commute
    with the positive per-row normalization), then margin loss + accuracy
    (sp > sn is equivalent to the argmax form for distinct maxima).

Performance notes (TimelineSim + HW-validated instruction set):
  - fp16 keys halve streamed bytes; DMA floor ~57us/core at the modeled
    360 B/ns aggregate DMA bandwidth. fp32 margin analysis: per-row
    |sim_pos - sim_neg| >= 0.034 vs ~1e-4 fp16-induced error.
  - One DMA per stream group (packed layout) keeps the shared descriptor
    generator (HWDGE, ~0.6us per DMA) off the critical path.
  - PE p-state warmup (one long zero accumulation group) covers the input
    DMA window so CNN and stream matmuls all dispatch at full clock; its
    PSUM tile doubles as stream super 0's, dodging the CNN-pool bank WAR.
  - Queue assignment: SP carries the input stream, ACT the collective
    chain (harmless: everything behind it needs the AllGather anyway) and
    outputs. Only HW-proven instructions are used (tensor_mask_reduce /
    tensor_tensor_reduce / gpsimd ALU ops all fail neuronxcc on TRN2).
"""
import numpy as np
from contextlib import ExitStack

import concourse.bass as bass
import concourse.tile as tile
from concourse import bacc, mybir
from concourse.bass_utils import run_bass_kernel_spmd

# ---- problem dims (hardcoded; harness passes matching inputs) ----
B, L = 128, 64
V, D = 25000, 300
C = 1000
KN = 100
KSIZES = (3, 4, 5)
M, KEY = 262144, 300
MARGIN = 0.1

N_CORES = 8
BPC = B // N_CORES          # batch rows per core
TOK = BPC * L               # tokens per core
DCN = 3                     # d-chunks of 100
DCW = 100                   # d-chunk width
CHUNK = 512                 # sim columns per PSUM bank
BLK = 16                    # label-pure block width
NCH = 66                    # chunks per core
W = NCH * CHUNK             # slab columns per core (33792)
CAP = N_CORES * W           # padded memory size (270336)
NBLK = W // BLK             # blocks per core (2112)
CPB = CHUNK // BLK          # blocks per chunk (32)
BIG = 16.0                  # mask sentinel; |sims_u| <= ~8 so +-16 acts as +-inf

SW = 2                      # chunks per super-chunk (PSUM/ACT granularity)
FINE = 2                    # trailing chunks run chunk-granular (short tail)
NSUP = (NCH - FINE) // SW   # wide supers per core (32)
SUPB = SW * CPB             # blocks per super (64)
# masked-min partials folded every FS supers (fewer DVE ops) with batched
# segmented max-reduces; batches shrink toward the end for a short tail
FS = 2                                        # supers per fold segment
NSEG = NSUP // FS + 1                         # segments (16 wide + fine)
RED_BATCH = (2, 2, 2, 2, 2, 2, 2, 2, 1)       # in segments, sum = NSEG

f32 = mybir.dt.float32
f16 = mybir.dt.float16

_CACHED_NC = None


def _group_sizes(g, taper=(1, 1, 1)):
    left = NSUP - sum(taper)
    sizes = []
    while left > 0:
        s = min(g, left)
        sizes.append(s)
        left -= s
    return sizes + list(taper)


def build(collective=True, g=2, ktbufs=6, scbufs=6, warmup=15, warmw=256):
    nc = bacc.Bacc("TRN2", target_bir_lowering=False, debug=False,
                   num_devices=N_CORES if collective else 1)
    qt_in = None
    if not collective:
        qt_in = nc.declare_dram_parameter("qtin", [DCW, DCN * B], f16,
                                          isOutput=False)

    ktg_in = nc.declare_dram_parameter("ktg", [DCW, NCH * DCN * CHUNK], f16,
                                       isOutput=False)
    et_in = nc.declare_dram_parameter("et", [DCW, DCN * TOK], f16,
                                      isOutput=False)
    wt_in = nc.declare_dram_parameter("wt", [DCW, sum(KSIZES) * DCN * KN], f16,
                                      isOutput=False)
    bias_in = nc.declare_dram_parameter("bias", [KN, len(KSIZES)], f32,
                                        isOutput=False)
    y_in = nc.declare_dram_parameter("yv", [B, 1], f32, isOutput=False)
    blab_in = nc.declare_dram_parameter("blab", [1, NBLK], f16, isOutput=False)

    pn_out = nc.declare_dram_parameter("pn", [B, 2], f32, isOutput=True)
    ss_out = nc.declare_dram_parameter("ss", [1, B], f32, isOutput=True)

    cc_in = nc.dram_tensor("cc_in", [DCW, DCN * BPC], f16)
    cc_out = nc.dram_tensor("cc_out", [N_CORES, DCW, DCN * BPC], f16,
                            addr_space="Shared")

    woff = {}   # column offset of each kernel size's weights in wt
    off = 0
    for k in KSIZES:
        woff[k] = off
        off += k * DCN * KN

    # reduce-batch boundaries: after these segment indices (fine = last)
    nseg = NSEG
    segb = FS * SUPB                             # blocks per wide segment
    assert sum(RED_BATCH) == nseg
    batch_after = {}
    acc = 0
    for bk in RED_BATCH:
        acc += bk
        batch_after[acc - 1] = (acc - bk, bk)    # last seg -> (start, size)

    with tile.TileContext(nc) as tc, ExitStack() as ctx:
        singles = ctx.enter_context(tc.tile_pool(name="singles", bufs=1))
        ktp = ctx.enter_context(tc.tile_pool(name="ktp", bufs=ktbufs))
        scp = ctx.enter_context(tc.tile_pool(name="scp", bufs=scbufs))

        # ------------- input DMAs (SP queue; no long waits) ---------------
        # split so the k=3 conv can start as soon as its operands land
        qtall = singles.tile([DCW, DCN * B], f16, tag="qtall")
        et = singles.tile([DCW, DCN * TOK], f16, tag="et")
        nc.sync.dma_start(out=et[:, :TOK], in_=et_in[:, :TOK])
        wsplit = KSIZES[0] * DCN * KN
        wt = singles.tile([DCW, sum(KSIZES) * DCN * KN], f16, tag="wt")
        nc.sync.dma_start(out=wt[:, :wsplit], in_=wt_in[:, :wsplit])
        nc.sync.dma_start(out=et[:, TOK:], in_=et_in[:, TOK:])
        nc.sync.dma_start(out=wt[:, wsplit:], in_=wt_in[:, wsplit:])
        bia = singles.tile([KN, len(KSIZES)], f32, tag="bias")
        nc.sync.dma_start(out=bia, in_=bias_in[:, :])
        if not collective:
            # no collective: queries come straight from DRAM; early in the
            # queue so the stream can start the moment the CNN finishes
            nc.sync.dma_start(out=qtall, in_=qt_in[:, :])
        y0 = singles.tile([B, 1], f32, tag="y0")
        nc.sync.dma_start(out=y0, in_=y_in[:, :])
        # block labels broadcast to all partitions via stride-0 DMA read
        blabB = singles.tile([B, NBLK], f16, tag="blabB")
        nc.sync.dma_start(out=blabB, in_=bass.AP(
            tensor=blab_in.ap().tensor, offset=0, ap=[[0, B], [1, NBLK]]))

        # ---------------- mask prep (overlaps CNN; DVE idle then) ---------
        eq01 = singles.tile([B, NBLK], f16, tag="eq01")
        nc.vector.tensor_scalar(out=eq01[:], in0=blabB[:], scalar1=y0[:],
                                scalar2=None, op0=mybir.AluOpType.is_equal)
        # eqp = +BIG where label==y else -BIG ; eqn = the opposite
        eqp = singles.tile([B, NBLK], f16, tag="eqp")
        nc.vector.tensor_scalar(out=eqp[:], in0=eq01[:], scalar1=2.0 * BIG,
                                scalar2=-BIG, op0=mybir.AluOpType.mult,
                                op1=mybir.AluOpType.add)
        eqn = singles.tile([B, NBLK], f16, tag="eqn")
        nc.vector.tensor_scalar(out=eqn[:], in0=eq01[:], scalar1=-2.0 * BIG,
                                scalar2=BIG, op0=mybir.AluOpType.mult,
                                op1=mybir.AluOpType.add)

        # ---------------- CNN phase ----------------
        feats = {}  # per kernel size: [100, BPC] f32 (this IS a qT d-chunk)
        auxps = ctx.enter_context(tc.tile_pool(name="auxps", bufs=1,
                                               space="PSUM"))
        with tc.tile_pool(name="cnnps", bufs=3, space="PSUM") as cnnps, \
             tc.tile_pool(name="cnnsb", bufs=4) as cnnsb:
            half = BPC // 2
            if warmup:
                # PE p-state warmup: one long zero accumulation group that
                # runs while the et/wt DMAs land, so CNN matmuls start at
                # full clock. The warm PSUM tile doubles as stream super
                # 0's tile, dodging the CNN pools' bank-reuse WAR.
                wz1 = singles.tile([1, B], f16, tag="wz1")
                nc.vector.memset(wz1, 0.0)
                wz2 = singles.tile([1, warmw], f16, tag="wz2")
                nc.vector.memset(wz2, 0.0)
                wps = auxps.tile([B, SW * CHUNK], f32, tag="warm")
                for i in range(warmup):
                    nc.tensor.matmul(wps[:, 0:warmw], wz1[:], wz2[:],
                                     start=(i == 0), stop=(i == warmup - 1))
            for ki, k in enumerate(KSIZES):
                lout = L - k + 1
                fk = singles.tile([KN, BPC], f32, tag=f"feats{k}")
                feats[k] = fk
                pss = [cnnps.tile([KN, half * lout], f32,
                                  name=f"cnnpsum{k}_{h}", tag="cnnpsum")
                       for h in range(2)]
                # dc-outer so the first matmuls only need the first et third;
                # h interleaved so PE stays busy while later thirds land
                for dc in range(DCN):
                    for h in range(2):
                        rhs_full = et[:, dc * TOK:(dc + 1) * TOK].rearrange(
                            "p (b l) -> p b l", l=L)
                        for t in range(k):
                            nc.tensor.matmul(
                                pss[h][:],
                                wt[:, woff[k] + (t * DCN + dc) * KN:
                                   woff[k] + (t * DCN + dc + 1) * KN],
                                rhs_full[:, h * half:(h + 1) * half,
                                         t:t + lout],
                                start=(dc == 0 and t == 0),
                                stop=(dc == DCN - 1 and t == k - 1))
                for h in range(2):
                    # bias + relu (ACT), then maxpool over positions (DVE)
                    rk = cnnsb.tile([KN, half * lout], f32, tag="relu")
                    nc.scalar.activation(rk[:], pss[h][:],
                                         mybir.ActivationFunctionType.Relu,
                                         bias=bia[:, ki:ki + 1], scale=1.0)
                    nc.vector.tensor_reduce(
                        out=fk[:, h * half:(h + 1) * half],
                        in_=rk.rearrange("p (b l) -> p b l", l=lout),
                        axis=mybir.AxisListType.X, op=mybir.AluOpType.max)

            # qT assembly: features in f16, AllGathered across cores. The
            # collective chain rides the ACT queue: everything emitted
            # after it on that queue depends on the AllGather anyway.
            if collective:
                fall = singles.tile([DCW, DCN * BPC], f16, tag="fall")
                for i, k in enumerate(KSIZES):
                    nc.vector.tensor_copy(
                        fall[:, i * BPC:(i + 1) * BPC], feats[k][:, :])
                nc.scalar.dma_start(out=cc_in[:, :], in_=fall[:])
                nc.gpsimd.collective_compute(
                    "AllGather", mybir.AluOpType.bypass,
                    replica_groups=[list(range(N_CORES))],
                    ins=[cc_in[:, :]], outs=[cc_out[:, :, :]])
                # qtall[p, dc*B + core*BPC + i] = cc_out[core, p, dc*BPC + i]
                src = bass.AP(
                    tensor=cc_out.ap().tensor, offset=0,
                    ap=[[DCN * BPC, DCW], [BPC, DCN],
                        [DCW * DCN * BPC, N_CORES], [1, BPC]])
                nc.scalar.dma_start(
                    out=qtall.rearrange("p (dc core i) -> p dc core i",
                                        dc=DCN, core=N_CORES),
                    in_=src)

        # ---------------- memory stream ----------------
        # PE fills two-bank PSUM supers; ACT copies them to f16; DVE turns
        # each super into 16-wide block maxes with a 4-level pairwise-max
        # tree (tensor_tensor gets DVE 2x mode, tensor_reduce does not).
        bmall = singles.tile([B, NBLK], f16, tag="bmall")
        t1 = singles.tile([B, SW * CHUNK // 2], f16, tag="t1")
        t2 = singles.tile([B, SW * CHUNK // 4], f16, tag="t2")
        t3 = singles.tile([B, SW * CHUNK // 8], f16, tag="t3")
        pmp = singles.tile([B, NBLK], f16, tag="pmp")
        pmn = singles.tile([B, NBLK], f16, tag="pmn")
        parts_p = singles.tile([B, nseg], f32, tag="parts_p")
        parts_n = singles.tile([B, nseg], f32, tag="parts_n")
        pn = singles.tile([B, 2], f32, tag="pn")

        sizes = _group_sizes(g)
        starts = [sum(sizes[:i]) for i in range(len(sizes))]
        nwid = SW * DCN * CHUNK          # slab columns per super
        mx = mybir.AluOpType.max

        def block_tree(sc_ap, nb, bout):
            # pairwise-max tree: [B, nb*BLK] f16 -> [B, nb] block maxes
            v0 = sc_ap.rearrange("p (nb blk) -> p nb blk", blk=BLK)
            v1 = t1[:, :nb * 8].rearrange("p (nb blk) -> p nb blk", blk=8)
            nc.vector.tensor_tensor(out=v1, in0=v0[:, :, 0:8],
                                    in1=v0[:, :, 8:16], op=mx)
            v2 = t2[:, :nb * 4].rearrange("p (nb blk) -> p nb blk", blk=4)
            nc.vector.tensor_tensor(out=v2, in0=v1[:, :, 0:4],
                                    in1=v1[:, :, 4:8], op=mx)
            v3 = t3[:, :nb * 2].rearrange("p (nb blk) -> p nb blk", blk=2)
            nc.vector.tensor_tensor(out=v3, in0=v2[:, :, 0:2],
                                    in1=v2[:, :, 2:4], op=mx)
            nc.vector.tensor_tensor(out=bout, in0=v3[:, :, 0:1].rearrange(
                "p nb one -> p (nb one)"), in1=v3[:, :, 1:2].rearrange(
                "p nb one -> p (nb one)"), op=mx)

        def fold_seg(si, blocks):
            # masked segment mins; batched segmented max-reduce over
            # equal-width wide segments (the fine segment reduces alone)
            b0b, b1b = si * segb, si * segb + blocks
            sl = slice(b0b, b1b)
            nc.vector.tensor_tensor(out=pmp[:, sl], in0=bmall[:, sl],
                                    in1=eqp[:, sl], op=mybir.AluOpType.min)
            nc.vector.tensor_tensor(out=pmn[:, sl], in0=bmall[:, sl],
                                    in1=eqn[:, sl], op=mybir.AluOpType.min)
            ba = batch_after.get(si)
            if ba is None:
                return
            s0_, bk = ba
            for pm_, parts in ((pmp, parts_p), (pmn, parts_n)):
                if si == nseg - 1:      # fine segment: irregular width
                    nc.vector.tensor_reduce(
                        out=parts[:, si:si + 1], in_=pm_[:, sl],
                        axis=mybir.AxisListType.X, op=mx)
                    continue
                nc.vector.tensor_reduce(
                    out=parts[:, s0_:s0_ + bk],
                    in_=pm_[:, s0_ * segb:(s0_ + bk) * segb].rearrange(
                        "p (k s) -> p k s", s=segb),
                    axis=mybir.AxisListType.X, op=mx)

        with tc.tile_pool(name="simps", bufs=3, space="PSUM") as simps:
            # feature norms for ALL batch rows, straight from the gathered
            # qT (gated only on qtall): the f16 feature quantization is
            # baked into the sims, so this is the consistent normalizer.
            ones = singles.tile([DCW, 1], f32, tag="ones")
            nc.vector.memset(ones, 1.0)
            sq2 = singles.tile([DCW, DCN * B], f32, tag="sq2")
            nc.vector.tensor_mul(sq2[:], qtall[:], qtall[:])
            # reuses the warm bank (WAR on super 0's ACT copy — fine, late)
            ssps = auxps.tile([1, B], f32, name="ssps", tag="warm")
            for dc in range(DCN):
                nc.tensor.matmul(ssps[:], ones[:],
                                 sq2[:, dc * B:(dc + 1) * B],
                                 start=(dc == 0), stop=(dc == DCN - 1))
            ss_sb = singles.tile([1, B], f32, tag="ss_sb")
            nc.vector.tensor_copy(ss_sb[:], ssps[:])
            nc.scalar.dma_start(out=ss_out[:, :], in_=ss_sb[:])

            # wide region: super-granular groups
            for gi, (s0, gsz) in enumerate(zip(starts, sizes)):
                kt = ktp.tile([DCW, g * nwid], f16, tag="kt")
                gw = gsz * nwid
                nc.sync.dma_start(
                    out=kt[:, :gw], in_=ktg_in[:, s0 * nwid:s0 * nwid + gw])
                for sl in range(gsz):
                    s = s0 + sl
                    if s == 0 and warmup:
                        ps = wps      # warm tile: no CNN-pool bank WAR
                    else:
                        ps = simps.tile([B, SW * CHUNK], f32, name="simpsum",
                                        tag="simpsum")
                    for sub in range(SW):
                        for dc in range(DCN):
                            nc.tensor.matmul(
                                ps[:, sub * CHUNK:(sub + 1) * CHUNK],
                                qtall[:, dc * B:(dc + 1) * B],
                                kt[:, ((sl * SW + sub) * DCN + dc) * CHUNK:
                                   ((sl * SW + sub) * DCN + dc + 1) * CHUNK],
                                start=(dc == 0), stop=(dc == DCN - 1))
                    sc = scp.tile([B, SW * CHUNK], f16, name="scw",
                                  tag="simf16")
                    nc.scalar.copy(sc[:], ps[:])
                    block_tree(sc[:], SUPB, bmall[:, s * SUPB:(s + 1) * SUPB])
                    if (s + 1) % FS == 0:
                        fold_seg(s // FS, segb)
            # fine tail: the last two chunks form a pseudo-super fed by two
            # chunk-granular DMAs, so the closing DMA->fold chain is short
            # separate PSUM tiles per fine chunk: a shared tile would make
            # chunk 2's start=True matmul false-WAR on chunk 1's ACT copy
            # (Tile tracks deps at tile granularity)
            psf1 = auxps.tile([B, SW * CHUNK], f32, name="psf1", tag="warm")
            psf2 = simps.tile([B, SW * CHUNK], f32, name="psf2", tag="simpsum")
            scf = scp.tile([B, SW * CHUNK], f16, name="scf", tag="simf16")
            for i, psf in enumerate((psf1, psf2)):
                c = NCH - FINE + i
                ktc = ktp.tile([DCW, g * nwid], f16, name="ktc", tag="kt")
                nc.sync.dma_start(
                    out=ktc[:, :DCN * CHUNK],
                    in_=ktg_in[:, c * DCN * CHUNK:(c + 1) * DCN * CHUNK])
                for dc in range(DCN):
                    nc.tensor.matmul(
                        psf[:, 0:CHUNK],
                        qtall[:, dc * B:(dc + 1) * B],
                        ktc[:, dc * CHUNK:(dc + 1) * CHUNK],
                        start=(dc == 0), stop=(dc == DCN - 1))
                nc.scalar.copy(scf[:, i * CHUNK:(i + 1) * CHUNK],
                               psf[:, 0:CHUNK])
            block_tree(scf[:], SUPB, bmall[:, NSUP * SUPB:])
            fold_seg(nseg - 1, FINE * CPB)

        # final: max over segment partials -> pos/neg. ONE output DMA: the
        # closing drain waits for the last DMA sem, and a second DMA costs
        # a serial HWDGE pass (~0.5us) more than the early-pos overlap buys
        nc.vector.tensor_reduce(out=pn[:, 0:1], in_=parts_p[:],
                                axis=mybir.AxisListType.X, op=mx)
        nc.vector.tensor_reduce(out=pn[:, 1:2], in_=parts_n[:],
                                axis=mybir.AxisListType.X, op=mx)
        nc.scalar.dma_start(out=pn_out[:, :], in_=pn[:])

    nc.compile()
    return nc


def _prep(x, y, embed, conv_w3, conv_b3, conv_w4, conv_b4, conv_w5, conv_b5,
          mem_keys, mem_values):
    """Host-side sharding/packing. Returns per-core input maps + combine data."""
    x = np.asarray(x)
    y64 = np.asarray(y).astype(np.int64)
    mv = np.asarray(mem_values).astype(np.int64)
    mk = np.asarray(mem_keys, dtype=np.float32)

    # --- label-sorted, block-pure padded permutation of the memory bank ---
    order = np.argsort(mv, kind="stable")
    cnt = np.bincount(mv, minlength=C)
    assert cnt.min() > 0, "kernel assumes every class present in memory"
    cstarts = np.zeros(C + 1, np.int64)
    cstarts[1:] = np.cumsum(cnt)
    parts = []
    for c in range(C):
        grp = order[cstarts[c]:cstarts[c + 1]]
        padn = (-len(grp)) % BLK
        if padn:
            grp = np.concatenate([grp, np.repeat(grp[0], padn)])
        parts.append(grp)
    perm = np.concatenate(parts)
    assert len(perm) <= CAP, f"padded size {len(perm)} exceeds CAP {CAP}"
    # tail pad duplicates the LAST class (keeps every class contiguous and
    # block-pure; the duplicates are real keys so maxes are exact)
    perm = np.concatenate([perm, np.repeat(parts[-1][0], CAP - len(perm))])
    labP = mv[perm]
    blab = labP[::BLK].astype(np.float16)          # [CAP // BLK]
    keysP = mk.astype(np.float16)[perm]            # cast before gather: half the traffic

    # --- embedding lookup (host gather; device gets ready eT slabs) ---
    emb16 = np.asarray(embed, dtype=np.float32).astype(np.float16)
    e = emb16[x]                                    # [B, L, 300]

    # --- conv weights packed into one tensor:
    #     wt[p, woff_k + (t*DCN+dc)*KN + kn] = w_k[kn, dc*100+p, t]
    wparts = []
    for k, w_ in ((3, conv_w3), (4, conv_w4), (5, conv_w5)):
        w_ = np.asarray(w_, dtype=np.float32)       # [KN, D, k]
        a = w_.reshape(KN, DCN, DCW, k).transpose(2, 3, 1, 0)  # [p, t, dc, kn]
        wparts.append(a.reshape(DCW, k * DCN * KN))
    wt = np.ascontiguousarray(np.concatenate(wparts, axis=1)).astype(np.float16)

    biases = np.zeros((KN, len(KSIZES)), np.float32)
    for i, b_ in enumerate((conv_b3, conv_b4, conv_b5)):
        biases[:, i] = np.asarray(b_, dtype=np.float32)

    yv = y64.astype(np.float32).reshape(B, 1)

    in_maps = []
    for c in range(N_CORES):
        # key slab packed chunk-major: ktg[p, (j*DCN+dc)*CHUNK + cc]
        #   = keysP[c*W + j*CHUNK + cc, dc*DCW + p]
        slab = keysP[c * W:(c + 1) * W]             # [W, 300]
        ktg = np.ascontiguousarray(
            slab.reshape(NCH, CHUNK, DCN, DCW)
            .transpose(3, 0, 2, 1)                  # [p, j, dc, cc]
            .reshape(DCW, NCH * DCN * CHUNK))
        # eT: et[p, dc*TOK + b*L + l] = e[c*BPC + b, l, dc*100 + p]
        ec = e[c * BPC:(c + 1) * BPC]               # [BPC, L, 300]
        et = np.ascontiguousarray(
            ec.reshape(BPC, L, DCN, DCW)
            .transpose(3, 2, 0, 1)                  # [p, dc, b, l]
            .reshape(DCW, DCN * TOK))
        m = {
            "ktg": ktg,
            "et": et,
            "wt": wt,
            "bias": biases,
            "yv": yv,
            "blab": np.ascontiguousarray(
                blab[c * NBLK:(c + 1) * NBLK]).reshape(1, NBLK),
        }
        in_maps.append(m)
    return in_maps, y64


def _combine(results, y64):
    pos = np.max([r["pn"][:, 0] for r in results], axis=0)
    neg = np.max([r["pn"][:, 1] for r in results], axis=0)
    ss = results[0]["ss"].reshape(B)    # every core computes all norms
    rn = 1.0 / np.maximum(np.sqrt(ss), 1e-12)
    sp = pos * rn
    sn = neg * rn
    loss = np.float32(np.mean(np.maximum(sn - sp + MARGIN, 0.0)))
    acc = np.float32(np.mean((sp > sn).astype(np.float32)))
    return loss, acc


def kernel(**inputs):
    global _CACHED_NC
    in_maps, y64 = _prep(**inputs)
    if _CACHED_NC is None:
        _CACHED_NC = build()
    res = run_bass_kernel_spmd(_CACHED_NC, in_maps,
                               core_ids=list(range(N_CORES)))
    return _combine(res.results, y64)


# revision 77
# speedup vs baseline: 1.9550x; 1.0020x over previous
"""Trainium2 Bass kernel for nn_CNN_Mem (CNN text encoder + cosine memory lookup).

Strategy (8 NeuronCores, SPMD):
  - Memory bank sharded along mem_size: host label-sorts mem_keys so every
    16-column block holds a single label (per-class groups padded by
    duplicating a real key of the same class; tail padded with the last
    class), casts to fp16, transposes to [300, M/8] slabs per core, and
    packs the three 100-row d-chunks chunk-major so each stream group is
    ONE contiguous DMA.
  - Each core: CNN for its 16 batch rows (embedding rows gathered host-side,
    convs as PSUM-accumulated matmuls over shifted APs, relu+bias on ACT,
    maxpool on DVE) -> feature chunks [100, 16] per kernel size = the
    d-chunks of q^T. AllGather across the 8 cores -> qT [100, 128] per
    d-chunk (single strided readback DMA).
  - Stream the packed key slab through the PE in [128, 1024] two-bank PSUM
    super-chunks (6 accumulated fp16 matmuls); ACT copies PSUM->SBUF f16;
    DVE computes per-block (16-wide) maxes with a 4-level pairwise-max
    tree (tensor_tensor runs in 2x mode, unlike tensor_reduce) -> bmall.
  - Masked maxes at block level: block labels are DMA-broadcast across
    partitions, compared against y (is_equal) and turned into +-BIG
    sentinels; per window, min(bmall, mask) + max-reduce give per-window
    sim_pos / sim_neg partials; a final max over windows produces pos/neg.
    Windows narrow toward the end of the stream (the last two chunks run
    chunk-granular) so the closing chain after the final matmul is short.
  - Feature norms come from the gathered qT itself (ones-matmul of its
    square), so the PE queue never waits on the CNN maxpool chain.
  - Host combines: max over cores, divide by feature norms (maxes commute
    with the positive per-row normalization), then margin loss + accuracy
    (sp > sn is equivalent to the argmax form for distinct maxima).

Performance notes (TimelineSim + HW-validated instruction set):
  - fp16 keys halve streamed bytes; DMA floor ~57us/core at the modeled
    360 B/ns aggregate DMA bandwidth. fp32 margin analysis: per-row
    |sim_pos - sim_neg| >= 0.034 vs ~1e-4 fp16-induced error.
  - One DMA per stream group (packed layout) keeps the shared descriptor
    generator (HWDGE, ~0.6us per DMA) off the critical path.
  - PE p-state warmup (one long zero accumulation group) covers the input
    DMA window so CNN and stream matmuls all dispatch at full clock; its
    PSUM tile doubles as stream super 0's, dodging the CNN-pool bank WAR.
  - Queue assignment: SP carries the input stream, ACT the collective
    chain (harmless: everything behind it needs the AllGather anyway) and
    outputs. Only HW-proven instructions are used (tensor_mask_reduce /
    tensor_tensor_reduce / gpsimd ALU ops all fail neuronxcc on TRN2).
"""
import numpy as np
from contextlib import ExitStack

import concourse.bass as bass
import concourse.tile as tile
from concourse import bacc, mybir
from concourse.bass_utils import run_bass_kernel_spmd

# ---- problem dims (hardcoded; harness passes matching inputs) ----
B, L = 128, 64
V, D = 25000, 300
C = 1000
KN = 100
KSIZES = (3, 4, 5)
M, KEY = 262144, 300
MARGIN = 0.1

N_CORES = 8
BPC = B // N_CORES          # batch rows per core
TOK = BPC * L               # tokens per core
DCN = 3                     # d-chunks of 100
DCW = 100                   # d-chunk width
CHUNK = 512                 # sim columns per PSUM bank
BLK = 16                    # label-pure block width
NCH = 66                    # chunks per core
W = NCH * CHUNK             # slab columns per core (33792)
CAP = N_CORES * W           # padded memory size (270336)
NBLK = W // BLK             # blocks per core (2112)
CPB = CHUNK // BLK          # blocks per chunk (32)
BIG = 16.0                  # mask sentinel; |sims_u| <= ~8 so +-16 acts as +-inf

SW = 2                      # chunks per super-chunk (PSUM/ACT granularity)
FINE = 2                    # trailing chunks run chunk-granular (short tail)
NSUP = (NCH - FINE) // SW   # wide supers per core (32)
SUPB = SW * CPB             # blocks per super (64)
# masked-min partials folded every FS supers (fewer DVE ops) with batched
# segmented max-reduces; batches shrink toward the end for a short tail
FS = 2                                        # supers per fold segment
NSEG = NSUP // FS + 1                         # segments (16 wide + fine)
RED_BATCH = (2, 2, 2, 2, 2, 2, 2, 2, 1)       # in segments, sum = NSEG

f32 = mybir.dt.float32
f16 = mybir.dt.float16

_CACHED_NC = None


def _group_sizes(g, taper=(1, 1, 1)):
    left = NSUP - sum(taper)
    sizes = []
    while left > 0:
        s = min(g, left)
        sizes.append(s)
        left -= s
    return sizes + list(taper)


def build(collective=True, g=2, ktbufs=6, scbufs=6, warmup=14, warmw=256):
    nc = bacc.Bacc("TRN2", target_bir_lowering=False, debug=False,
                   num_devices=N_CORES if collective else 1)
    qt_in = None
    if not collective:
        qt_in = nc.declare_dram_parameter("qtin", [DCW, DCN * B], f16,
                                          isOutput=False)

    ktg_in = nc.declare_dram_parameter("ktg", [DCW, NCH * DCN * CHUNK], f16,
                                       isOutput=False)
    et_in = nc.declare_dram_parameter("et", [DCW, DCN * TOK], f16,
                                      isOutput=False)
    wt_in = nc.declare_dram_parameter("wt", [DCW, sum(KSIZES) * DCN * KN], f16,
                                      isOutput=False)
    bias_in = nc.declare_dram_parameter("bias", [KN, len(KSIZES)], f32,
                                        isOutput=False)
    y_in = nc.declare_dram_parameter("yv", [B, 1], f32, isOutput=False)
    blab_in = nc.declare_dram_parameter("blab", [1, NBLK], f16, isOutput=False)

    pn_out = nc.declare_dram_parameter("pn", [B, 2], f32, isOutput=True)
    ss_out = nc.declare_dram_parameter("ss", [1, B], f32, isOutput=True)

    cc_in = nc.dram_tensor("cc_in", [DCW, DCN * BPC], f16)
    cc_out = nc.dram_tensor("cc_out", [N_CORES, DCW, DCN * BPC], f16,
                            addr_space="Shared")

    woff = {}   # column offset of each kernel size's weights in wt
    off = 0
    for k in KSIZES:
        woff[k] = off
        off += k * DCN * KN

    # reduce-batch boundaries: after these segment indices (fine = last)
    nseg = NSEG
    segb = FS * SUPB                             # blocks per wide segment
    assert sum(RED_BATCH) == nseg
    batch_after = {}
    acc = 0
    for bk in RED_BATCH:
        acc += bk
        batch_after[acc - 1] = (acc - bk, bk)    # last seg -> (start, size)

    with tile.TileContext(nc) as tc, ExitStack() as ctx:
        singles = ctx.enter_context(tc.tile_pool(name="singles", bufs=1))
        ktp = ctx.enter_context(tc.tile_pool(name="ktp", bufs=ktbufs))
        scp = ctx.enter_context(tc.tile_pool(name="scp", bufs=scbufs))

        # ------------- input DMAs (SP queue; no long waits) ---------------
        # split so the k=3 conv can start as soon as its operands land
        qtall = singles.tile([DCW, DCN * B], f16, tag="qtall")
        et = singles.tile([DCW, DCN * TOK], f16, tag="et")
        nc.sync.dma_start(out=et[:, :TOK], in_=et_in[:, :TOK])
        wsplit = KSIZES[0] * DCN * KN
        wt = singles.tile([DCW, sum(KSIZES) * DCN * KN], f16, tag="wt")
        nc.sync.dma_start(out=wt[:, :wsplit], in_=wt_in[:, :wsplit])
        nc.sync.dma_start(out=et[:, TOK:], in_=et_in[:, TOK:])
        nc.sync.dma_start(out=wt[:, wsplit:], in_=wt_in[:, wsplit:])
        bia = singles.tile([KN, len(KSIZES)], f32, tag="bias")
        nc.sync.dma_start(out=bia, in_=bias_in[:, :])
        if not collective:
            # no collective: queries come straight from DRAM; early in the
            # queue so the stream can start the moment the CNN finishes
            nc.sync.dma_start(out=qtall, in_=qt_in[:, :])
        y0 = singles.tile([B, 1], f32, tag="y0")
        nc.sync.dma_start(out=y0, in_=y_in[:, :])
        # block labels broadcast to all partitions via stride-0 DMA read
        blabB = singles.tile([B, NBLK], f16, tag="blabB")
        nc.sync.dma_start(out=blabB, in_=bass.AP(
            tensor=blab_in.ap().tensor, offset=0, ap=[[0, B], [1, NBLK]]))

        # ---------------- mask prep (overlaps CNN; DVE idle then) ---------
        eq01 = singles.tile([B, NBLK], f16, tag="eq01")
        nc.vector.tensor_scalar(out=eq01[:], in0=blabB[:], scalar1=y0[:],
                                scalar2=None, op0=mybir.AluOpType.is_equal)
        # eqp = +BIG where label==y else -BIG ; eqn = the opposite
        eqp = singles.tile([B, NBLK], f16, tag="eqp")
        nc.vector.tensor_scalar(out=eqp[:], in0=eq01[:], scalar1=2.0 * BIG,
                                scalar2=-BIG, op0=mybir.AluOpType.mult,
                                op1=mybir.AluOpType.add)
        eqn = singles.tile([B, NBLK], f16, tag="eqn")
        nc.vector.tensor_scalar(out=eqn[:], in0=eq01[:], scalar1=-2.0 * BIG,
                                scalar2=BIG, op0=mybir.AluOpType.mult,
                                op1=mybir.AluOpType.add)

        # ---------------- CNN phase ----------------
        feats = {}  # per kernel size: [100, BPC] f32 (this IS a qT d-chunk)
        auxps = ctx.enter_context(tc.tile_pool(name="auxps", bufs=1,
                                               space="PSUM"))
        with tc.tile_pool(name="cnnps", bufs=3, space="PSUM") as cnnps, \
             tc.tile_pool(name="cnnsb", bufs=4) as cnnsb:
            half = BPC // 2
            if warmup:
                # PE p-state warmup: one long zero accumulation group that
                # runs while the et/wt DMAs land, so CNN matmuls start at
                # full clock. The warm PSUM tile doubles as stream super
                # 0's tile, dodging the CNN pools' bank-reuse WAR.
                wz1 = singles.tile([1, B], f16, tag="wz1")
                nc.vector.memset(wz1, 0.0)
                wz2 = singles.tile([1, warmw], f16, tag="wz2")
                nc.vector.memset(wz2, 0.0)
                wps = auxps.tile([B, SW * CHUNK], f32, tag="warm")
                for i in range(warmup):
                    nc.tensor.matmul(wps[:, 0:warmw], wz1[:], wz2[:],
                                     start=(i == 0), stop=(i == warmup - 1))
            for ki, k in enumerate(KSIZES):
                lout = L - k + 1
                fk = singles.tile([KN, BPC], f32, tag=f"feats{k}")
                feats[k] = fk
                pss = [cnnps.tile([KN, half * lout], f32,
                                  name=f"cnnpsum{k}_{h}", tag="cnnpsum")
                       for h in range(2)]
                # dc-outer so the first matmuls only need the first et third;
                # h interleaved so PE stays busy while later thirds land
                for dc in range(DCN):
                    for h in range(2):
                        rhs_full = et[:, dc * TOK:(dc + 1) * TOK].rearrange(
                            "p (b l) -> p b l", l=L)
                        for t in range(k):
                            nc.tensor.matmul(
                                pss[h][:],
                                wt[:, woff[k] + (t * DCN + dc) * KN:
                                   woff[k] + (t * DCN + dc + 1) * KN],
                                rhs_full[:, h * half:(h + 1) * half,
                                         t:t + lout],
                                start=(dc == 0 and t == 0),
                                stop=(dc == DCN - 1 and t == k - 1))
                for h in range(2):
                    # bias + relu (ACT), then maxpool over positions (DVE)
                    rk = cnnsb.tile([KN, half * lout], f32, tag="relu")
                    nc.scalar.activation(rk[:], pss[h][:],
                                         mybir.ActivationFunctionType.Relu,
                                         bias=bia[:, ki:ki + 1], scale=1.0)
                    nc.vector.tensor_reduce(
                        out=fk[:, h * half:(h + 1) * half],
                        in_=rk.rearrange("p (b l) -> p b l", l=lout),
                        axis=mybir.AxisListType.X, op=mybir.AluOpType.max)

            # qT assembly: features in f16, AllGathered across cores. The
            # collective chain rides the ACT queue: everything emitted
            # after it on that queue depends on the AllGather anyway.
            if collective:
                fall = singles.tile([DCW, DCN * BPC], f16, tag="fall")
                for i, k in enumerate(KSIZES):
                    nc.vector.tensor_copy(
                        fall[:, i * BPC:(i + 1) * BPC], feats[k][:, :])
                nc.scalar.dma_start(out=cc_in[:, :], in_=fall[:])
                nc.gpsimd.collective_compute(
                    "AllGather", mybir.AluOpType.bypass,
                    replica_groups=[list(range(N_CORES))],
                    ins=[cc_in[:, :]], outs=[cc_out[:, :, :]])
                # qtall[p, dc*B + core*BPC + i] = cc_out[core, p, dc*BPC + i]
                src = bass.AP(
                    tensor=cc_out.ap().tensor, offset=0,
                    ap=[[DCN * BPC, DCW], [BPC, DCN],
                        [DCW * DCN * BPC, N_CORES], [1, BPC]])
                nc.scalar.dma_start(
                    out=qtall.rearrange("p (dc core i) -> p dc core i",
                                        dc=DCN, core=N_CORES),
                    in_=src)

        # ---------------- memory stream ----------------
        # PE fills two-bank PSUM supers; ACT copies them to f16; DVE turns
        # each super into 16-wide block maxes with a 4-level pairwise-max
        # tree (tensor_tensor gets DVE 2x mode, tensor_reduce does not).
        bmall = singles.tile([B, NBLK], f16, tag="bmall")
        t1 = singles.tile([B, SW * CHUNK // 2], f16, tag="t1")
        t2 = singles.tile([B, SW * CHUNK // 4], f16, tag="t2")
        t3 = singles.tile([B, SW * CHUNK // 8], f16, tag="t3")
        pmp = singles.tile([B, NBLK], f16, tag="pmp")
        pmn = singles.tile([B, NBLK], f16, tag="pmn")
        parts_p = singles.tile([B, nseg], f32, tag="parts_p")
        parts_n = singles.tile([B, nseg], f32, tag="parts_n")
        pn = singles.tile([B, 2], f32, tag="pn")

        sizes = _group_sizes(g)
        starts = [sum(sizes[:i]) for i in range(len(sizes))]
        nwid = SW * DCN * CHUNK          # slab columns per super
        mx = mybir.AluOpType.max

        def block_tree(sc_ap, nb, bout):
            # pairwise-max tree: [B, nb*BLK] f16 -> [B, nb] block maxes
            v0 = sc_ap.rearrange("p (nb blk) -> p nb blk", blk=BLK)
            v1 = t1[:, :nb * 8].rearrange("p (nb blk) -> p nb blk", blk=8)
            nc.vector.tensor_tensor(out=v1, in0=v0[:, :, 0:8],
                                    in1=v0[:, :, 8:16], op=mx)
            v2 = t2[:, :nb * 4].rearrange("p (nb blk) -> p nb blk", blk=4)
            nc.vector.tensor_tensor(out=v2, in0=v1[:, :, 0:4],
                                    in1=v1[:, :, 4:8], op=mx)
            v3 = t3[:, :nb * 2].rearrange("p (nb blk) -> p nb blk", blk=2)
            nc.vector.tensor_tensor(out=v3, in0=v2[:, :, 0:2],
                                    in1=v2[:, :, 2:4], op=mx)
            nc.vector.tensor_tensor(out=bout, in0=v3[:, :, 0:1].rearrange(
                "p nb one -> p (nb one)"), in1=v3[:, :, 1:2].rearrange(
                "p nb one -> p (nb one)"), op=mx)

        def fold_seg(si, blocks):
            # masked segment mins; batched segmented max-reduce over
            # equal-width wide segments (the fine segment reduces alone)
            b0b, b1b = si * segb, si * segb + blocks
            sl = slice(b0b, b1b)
            nc.vector.tensor_tensor(out=pmp[:, sl], in0=bmall[:, sl],
                                    in1=eqp[:, sl], op=mybir.AluOpType.min)
            nc.vector.tensor_tensor(out=pmn[:, sl], in0=bmall[:, sl],
                                    in1=eqn[:, sl], op=mybir.AluOpType.min)
            ba = batch_after.get(si)
            if ba is None:
                return
            s0_, bk = ba
            for pm_, parts in ((pmp, parts_p), (pmn, parts_n)):
                if si == nseg - 1:      # fine segment: irregular width
                    nc.vector.tensor_reduce(
                        out=parts[:, si:si + 1], in_=pm_[:, sl],
                        axis=mybir.AxisListType.X, op=mx)
                    continue
                nc.vector.tensor_reduce(
                    out=parts[:, s0_:s0_ + bk],
                    in_=pm_[:, s0_ * segb:(s0_ + bk) * segb].rearrange(
                        "p (k s) -> p k s", s=segb),
                    axis=mybir.AxisListType.X, op=mx)

        with tc.tile_pool(name="simps", bufs=3, space="PSUM") as simps:
            # feature norms for ALL batch rows, straight from the gathered
            # qT (gated only on qtall): the f16 feature quantization is
            # baked into the sims, so this is the consistent normalizer.
            ones = singles.tile([DCW, 1], f32, tag="ones")
            nc.vector.memset(ones, 1.0)
            sq2 = singles.tile([DCW, DCN * B], f32, tag="sq2")
            nc.vector.tensor_mul(sq2[:], qtall[:], qtall[:])
            # reuses the warm bank (WAR on super 0's ACT copy — fine, late)
            ssps = auxps.tile([1, B], f32, name="ssps", tag="warm")
            for dc in range(DCN):
                nc.tensor.matmul(ssps[:], ones[:],
                                 sq2[:, dc * B:(dc + 1) * B],
                                 start=(dc == 0), stop=(dc == DCN - 1))
            ss_sb = singles.tile([1, B], f32, tag="ss_sb")
            nc.vector.tensor_copy(ss_sb[:], ssps[:])
            nc.scalar.dma_start(out=ss_out[:, :], in_=ss_sb[:])

            # wide region: super-granular groups
            for gi, (s0, gsz) in enumerate(zip(starts, sizes)):
                kt = ktp.tile([DCW, g * nwid], f16, tag="kt")
                gw = gsz * nwid
                nc.sync.dma_start(
                    out=kt[:, :gw], in_=ktg_in[:, s0 * nwid:s0 * nwid + gw])
                for sl in range(gsz):
                    s = s0 + sl
                    if s == 0 and warmup:
                        ps = wps      # warm tile: no CNN-pool bank WAR
                    else:
                        ps = simps.tile([B, SW * CHUNK], f32, name="simpsum",
                                        tag="simpsum")
                    for sub in range(SW):
                        for dc in range(DCN):
                            nc.tensor.matmul(
                                ps[:, sub * CHUNK:(sub + 1) * CHUNK],
                                qtall[:, dc * B:(dc + 1) * B],
                                kt[:, ((sl * SW + sub) * DCN + dc) * CHUNK:
                                   ((sl * SW + sub) * DCN + dc + 1) * CHUNK],
                                start=(dc == 0), stop=(dc == DCN - 1))
                    sc = scp.tile([B, SW * CHUNK], f16, name="scw",
                                  tag="simf16")
                    nc.scalar.copy(sc[:], ps[:])
                    block_tree(sc[:], SUPB, bmall[:, s * SUPB:(s + 1) * SUPB])
                    if (s + 1) % FS == 0:
                        fold_seg(s // FS, segb)
            # fine tail: the last two chunks form a pseudo-super fed by two
            # chunk-granular DMAs, so the closing DMA->fold chain is short
            # separate PSUM tiles per fine chunk: a shared tile would make
            # chunk 2's start=True matmul false-WAR on chunk 1's ACT copy
            # (Tile tracks deps at tile granularity)
            psf1 = auxps.tile([B, SW * CHUNK], f32, name="psf1", tag="warm")
            psf2 = simps.tile([B, SW * CHUNK], f32, name="psf2", tag="simpsum")
            scf = scp.tile([B, SW * CHUNK], f16, name="scf", tag="simf16")
            for i, psf in enumerate((psf1, psf2)):
                c = NCH - FINE + i
                ktc = ktp.tile([DCW, g * nwid], f16, name="ktc", tag="kt")
                nc.sync.dma_start(
                    out=ktc[:, :DCN * CHUNK],
                    in_=ktg_in[:, c * DCN * CHUNK:(c + 1) * DCN * CHUNK])
                for dc in range(DCN):
                    nc.tensor.matmul(
                        psf[:, 0:CHUNK],
                        qtall[:, dc * B:(dc + 1) * B],
                        ktc[:, dc * CHUNK:(dc + 1) * CHUNK],
                        start=(dc == 0), stop=(dc == DCN - 1))
                nc.scalar.copy(scf[:, i * CHUNK:(i + 1) * CHUNK],
                               psf[:, 0:CHUNK])
            block_tree(scf[:], SUPB, bmall[:, NSUP * SUPB:])
            fold_seg(nseg - 1, FINE * CPB)

        # final: max over segment partials -> pos/neg. ONE output DMA: the
        # closing drain waits for the last DMA sem, and a second DMA costs
        # a serial HWDGE pass (~0.5us) more than the early-pos overlap buys
        nc.vector.tensor_reduce(out=pn[:, 0:1], in_=parts_p[:],
                                axis=mybir.AxisListType.X, op=mx)
        nc.vector.tensor_reduce(out=pn[:, 1:2], in_=parts_n[:],
                                axis=mybir.AxisListType.X, op=mx)
        # SP queue: its descriptor-gen latency is 134ns lower than ACT's
        nc.sync.dma_start(out=pn_out[:, :], in_=pn[:])

    nc.compile()
    return nc


def _prep(x, y, embed, conv_w3, conv_b3, conv_w4, conv_b4, conv_w5, conv_b5,
          mem_keys, mem_values):
    """Host-side sharding/packing. Returns per-core input maps + combine data."""
    x = np.asarray(x)
    y64 = np.asarray(y).astype(np.int64)
    mv = np.asarray(mem_values).astype(np.int64)
    mk = np.asarray(mem_keys, dtype=np.float32)

    # --- label-sorted, block-pure padded permutation of the memory bank ---
    order = np.argsort(mv, kind="stable")
    cnt = np.bincount(mv, minlength=C)
    assert cnt.min() > 0, "kernel assumes every class present in memory"
    cstarts = np.zeros(C + 1, np.int64)
    cstarts[1:] = np.cumsum(cnt)
    parts = []
    for c in range(C):
        grp = order[cstarts[c]:cstarts[c + 1]]
        padn = (-len(grp)) % BLK
        if padn:
            grp = np.concatenate([grp, np.repeat(grp[0], padn)])
        parts.append(grp)
    perm = np.concatenate(parts)
    assert len(perm) <= CAP, f"padded size {len(perm)} exceeds CAP {CAP}"
    # tail pad duplicates the LAST class (keeps every class contiguous and
    # block-pure; the duplicates are real keys so maxes are exact)
    perm = np.concatenate([perm, np.repeat(parts[-1][0], CAP - len(perm))])
    labP = mv[perm]
    blab = labP[::BLK].astype(np.float16)          # [CAP // BLK]
    keysP = mk.astype(np.float16)[perm]            # cast before gather: half the traffic

    # --- embedding lookup (host gather; device gets ready eT slabs) ---
    emb16 = np.asarray(embed, dtype=np.float32).astype(np.float16)
    e = emb16[x]                                    # [B, L, 300]

    # --- conv weights packed into one tensor:
    #     wt[p, woff_k + (t*DCN+dc)*KN + kn] = w_k[kn, dc*100+p, t]
    wparts = []
    for k, w_ in ((3, conv_w3), (4, conv_w4), (5, conv_w5)):
        w_ = np.asarray(w_, dtype=np.float32)       # [KN, D, k]
        a = w_.reshape(KN, DCN, DCW, k).transpose(2, 3, 1, 0)  # [p, t, dc, kn]
        wparts.append(a.reshape(DCW, k * DCN * KN))
    wt = np.ascontiguousarray(np.concatenate(wparts, axis=1)).astype(np.float16)

    biases = np.zeros((KN, len(KSIZES)), np.float32)
    for i, b_ in enumerate((conv_b3, conv_b4, conv_b5)):
        biases[:, i] = np.asarray(b_, dtype=np.float32)

    yv = y64.astype(np.float32).reshape(B, 1)

    in_maps = []
    for c in range(N_CORES):
        # key slab packed chunk-major: ktg[p, (j*DCN+dc)*CHUNK + cc]
        #   = keysP[c*W + j*CHUNK + cc, dc*DCW + p]
        slab = keysP[c * W:(c + 1) * W]             # [W, 300]
        ktg = np.ascontiguousarray(
            slab.reshape(NCH, CHUNK, DCN, DCW)
            .transpose(3, 0, 2, 1)                  # [p, j, dc, cc]
            .reshape(DCW, NCH * DCN * CHUNK))
        # eT: et[p, dc*TOK + b*L + l] = e[c*BPC + b, l, dc*100 + p]
        ec = e[c * BPC:(c + 1) * BPC]               # [BPC, L, 300]
        et = np.ascontiguousarray(
            ec.reshape(BPC, L, DCN, DCW)
            .transpose(3, 2, 0, 1)                  # [p, dc, b, l]
            .reshape(DCW, DCN * TOK))
        m = {
            "ktg": ktg,
            "et": et,
            "wt": wt,
            "bias": biases,
            "yv": yv,
            "blab": np.ascontiguousarray(
                blab[c * NBLK:(c + 1) * NBLK]).reshape(1, NBLK),
        }
        in_maps.append(m)
    return in_maps, y64


def _combine(results, y64):
    pos = np.max([r["pn"][:, 0] for r in results], axis=0)
    neg = np.max([r["pn"][:, 1] for r in results], axis=0)
    ss = results[0]["ss"].reshape(B)    # every core computes all norms
    rn = 1.0 / np.maximum(np.sqrt(ss), 1e-12)
    sp = pos * rn
    sn = neg * rn
    loss = np.float32(np.mean(np.maximum(sn - sp + MARGIN, 0.0)))
    acc = np.float32(np.mean((sp > sn).astype(np.float32)))
    return loss, acc


def kernel(**inputs):
    global _CACHED_NC
    in_maps, y64 = _prep(**inputs)
    if _CACHED_NC is None:
        _CACHED_NC = build()
    res = run_bass_kernel_spmd(_CACHED_NC, in_maps,
                               core_ids=list(range(N_CORES)))
    return _combine(res.results, y64)
